# revision 1
# baseline (speedup 1.0000x reference)
"""CosineAttention Trainium2 kernel (8 NeuronCores, SPMD).

Sharding: 16 (batch, head) pairs -> 8 cores, 2 heads (one batch) per core.
Per core, attention runs in transposed-score layout (scoresT[kpos, qpos]) so
both attention matmuls contract over the partition dim with no transposes:
  MM1: scoresT = kT.T-slice.T @ qT-slice        (K=32 head_dim)
  exp: ACT Exp with per-partition scale=(k pixel-norm * 1/sqrt(hd)) and
       bias=ln(v pixel-norm) folded in -> e = sv[k]*exp(true scoreT)
  MM2: lhsT = [v_hat | 1/sv] (M=33): accumulates y^T rows and the softmax
       denominator row in one PSUM accumulation group.
Weight-norm scales fold into conv PSUM evictions (per-partition scalars).
The out-conv is computed per-core on the core's 64 attention channels; the
8 partial results (each including 1/4 of the x residual term) are summed on
host per batch -- that sum is the gather/unshard step.
"""
import numpy as np
import concourse.bass as bass
import concourse.tile as tile
from concourse import mybir
from concourse.bass_utils import run_bass_kernel_spmd

F32 = mybir.dt.float32
AF = mybir.ActivationFunctionType
ALU = mybir.AluOpType

EPS = 1e-4
MP_T = 0.3
INV_SCALE = 1.0 / np.sqrt(MP_T ** 2 + (1.0 - MP_T) ** 2)
C = 256          # channels
HW = 4096        # pixels
HD = 32          # head dim
NCH = 8          # 512-wide pixel chunks
KT = 32          # 128-wide kpos tiles
LOG_ISQ_HD = float(np.log(1.0 / np.sqrt(HD)))
C_X = 0.25 * (1.0 - MP_T) * INV_SCALE     # per-core share of residual
C_Y = MP_T * INV_SCALE                    # folded into w_out scale
W_EPS = 16.0 * EPS                        # sqrt(fan_in)*EPS with fan_in=256


def _split_waits(nc):
    """This walrus accepts 1 sync wait per engine instruction: hoist extras
    into preceding NoOps on the same engine (engines are in-order)."""
    for f in nc.m.functions:
        for bb in f.blocks:
            newlist = []
            for inst in bb.instructions:
                si = inst.sync_info
                if si is not None and si.on_wait is not None and len(si.on_wait) > 1:
                    waits = list(si.on_wait)
                    if "DMA" in type(inst).__name__:
                        # keep the compute-engine sem on the DMA descriptor;
                        # hoist DMA-queue sems (monotonic, engine-stall safe)
                        hw = [w for w in waits if str(w.ant_name).startswith("DMA")]
                        eng = [w for w in waits if not str(w.ant_name).startswith("DMA")]
                        if eng:
                            keep, extra = eng[-1:], hw + eng[:-1]
                        else:
                            keep, extra = hw[-1:], hw[:-1]
                    else:
                        extra, keep = waits[:-1], waits[-1:]
                    for idx, w in enumerate(extra):
                        nop = mybir.InstNoOp(
                            name=f"{inst.name}_ws{idx}", ins=[], outs=[],
                            sync_info=mybir.SyncInfo(on_wait=[w], on_update=[]))
                        nop.engine = inst.engine
                        newlist.append(nop)
                    inst.sync_info = mybir.SyncInfo(
                        on_wait=keep, on_update=list(si.on_update or []))
                newlist.append(inst)
            bb.instructions = newlist


def _weight_scale_rows(nc, work, nat_ap, p):
    """Per-row weight-norm scale s = 1/(||w_row|| + 16*eps) for natural-layout
    [p, 256] weight rows. Returns a [p, 1] sbuf AP."""
    sq = work.tile([p, 256], F32, tag="wsq", name=f"wsq_{nc.next_id()}")
    nc.vector.tensor_mul(sq, nat_ap, nat_ap)
    ssq = work.tile([p, 1], F32, tag="wssq", name=f"wssq_{nc.next_id()}")
    nc.vector.tensor_reduce(ssq, sq, axis=mybir.AxisListType.X, op=ALU.add)
    ln = work.tile([p, 1], F32, tag="wln", name=f"wln_{nc.next_id()}")
    nc.scalar.activation(ln, ssq, AF.Ln, bias=0.0, scale=1.0)
    n = work.tile([p, 1], F32, tag="wn", name=f"wn_{nc.next_id()}")
    nc.scalar.activation(n, ln, AF.Exp, bias=0.0, scale=0.5)
    ne = work.tile([p, 1], F32, tag="wne", name=f"wne_{nc.next_id()}")
    nc.vector.tensor_scalar_add(ne, n, W_EPS)
    s = work.tile([p, 1], F32, tag="ws", name=f"ws_{nc.next_id()}")
    nc.vector.reciprocal(s, ne)
    return s


def build_program(split=True):
    nc = bass.Bass()
    x_d = nc.declare_dram_parameter("x", [C, HW], F32, isOutput=False)
    wqn_d = nc.declare_dram_parameter("wqn", [64, C], F32, isOutput=False)
    wkn_d = nc.declare_dram_parameter("wkn", [64, C], F32, isOutput=False)
    wqT_d = nc.declare_dram_parameter("wqT", [C, 64], F32, isOutput=False)
    wkT_d = nc.declare_dram_parameter("wkT", [C, 64], F32, isOutput=False)
    wvT_d = nc.declare_dram_parameter("wvT", [C, 64], F32, isOutput=False)
    won_d = nc.declare_dram_parameter("won", [C, C], F32, isOutput=False)
    woT4_d = nc.declare_dram_parameter("woT4", [128, C], F32, isOutput=False)
    y_d = nc.declare_dram_parameter("y", [C, HW], F32, isOutput=True)
    bq0_d = nc.dram_tensor("bq0", [32, 128], F32)
    bq1_d = nc.dram_tensor("bq1", [32, 128], F32)
    bqs = [bq0_d, bq1_d]

    with tile.TileContext(nc) as tc:
        with tc.tile_pool(name="singles", bufs=1) as sg, \
             tc.tile_pool(name="work", bufs=2) as work, \
             tc.tile_pool(name="scratch", bufs=2) as scr, \
             tc.tile_pool(name="epool", bufs=4) as ep, \
             tc.tile_pool(name="opool", bufs=4) as op, \
             tc.tile_pool(name="scps", bufs=1, space="PSUM") as scps, \
             tc.tile_pool(name="accps", bufs=4, space="PSUM") as accps, \
             tc.tile_pool(name="finps", bufs=2, space="PSUM") as finps:

            # ---------------- P0: loads ----------------
            x_sb = sg.tile([128, 2, HW], F32)
            nc.sync.dma_start(out=x_sb, in_=x_d[:].rearrange("(t p) f -> p t f", p=128))
            wqT_sb = sg.tile([128, 2, 64], F32)
            nc.sync.dma_start(out=wqT_sb, in_=wqT_d[:].rearrange("(t p) m -> p t m", p=128))
            wkT_sb = sg.tile([128, 2, 64], F32)
            nc.sync.dma_start(out=wkT_sb, in_=wkT_d[:].rearrange("(t p) m -> p t m", p=128))
            wvT_sb = sg.tile([128, 2, 64], F32)
            nc.sync.dma_start(out=wvT_sb, in_=wvT_d[:].rearrange("(t p) m -> p t m", p=128))
            wqn_sb = sg.tile([64, C], F32)
            nc.sync.dma_start(out=wqn_sb, in_=wqn_d[:])
            wkn_sb = sg.tile([64, C], F32)
            nc.sync.dma_start(out=wkn_sb, in_=wkn_d[:])
            won_sb = sg.tile([128, 2, C], F32)
            nc.sync.dma_start(out=won_sb, in_=won_d[:].rearrange("(t p) m -> p t m", p=128))
            woT4_sb = sg.tile([128, C], F32)
            nc.sync.dma_start(out=woT4_sb, in_=woT4_d[:])
            ones = sg.tile([128, 128], F32)
            nc.vector.memset(ones, 1.0)
            eps_col = sg.tile([128, 1], F32)
            nc.vector.memset(eps_col, EPS)
            lniq_col = sg.tile([128, 1], F32)
            nc.vector.memset(lniq_col, LOG_ISQ_HD)

            qT = sg.tile([64, HW], F32)
            kT = sg.tile([64, HW], F32)
            va = sg.tile([128, KT * 66], F32)   # per kt: v_h0(32)|sinv_h0|v_h1(32)|sinv_h1
            vbias = sg.tile([128, 64], F32)     # ln(sv), col = kt*2 + h
            skcol = sg.tile([128, 64], F32)     # exp scale, col = kt*2 + h
            sqrow = sg.tile([1, 2 * HW], F32)   # q norm scales row form per head

            # ---------------- P1: weight-norm scales ----------------
            sqq = _weight_scale_rows(nc, work, wqn_sb[:, :], 64)
            sqk = _weight_scale_rows(nc, work, wkn_sb[:, :], 64)
            swo = sg.tile([128, 2], F32)
            wosq = work.tile([128, 2, C], F32, tag="wosq")
            nc.vector.tensor_mul(wosq, won_sb, won_sb)
            wossq = work.tile([128, 2], F32, tag="wossq")
            nc.vector.tensor_reduce(wossq, wosq, axis=mybir.AxisListType.X, op=ALU.add)
            woln = work.tile([128, 2], F32, tag="woln")
            nc.scalar.activation(woln, wossq, AF.Ln, bias=0.0, scale=1.0)
            won_n = work.tile([128, 2], F32, tag="won_n")
            nc.scalar.activation(won_n, woln, AF.Exp, bias=0.0, scale=0.5)
            won_ne = work.tile([128, 2], F32, tag="won_ne")
            nc.vector.tensor_scalar_add(won_ne, won_n, W_EPS)
            swo_inv = work.tile([128, 2], F32, tag="swo_inv")
            nc.vector.reciprocal(swo_inv, won_ne)
            nc.vector.tensor_scalar_mul(swo, swo_inv, float(C_Y))

            # wv column scales: s_v[col] = 1/(||w_v[col]|| + 16 eps), fold into wvT
            wvsq = work.tile([128, 2, 64], F32, tag="wvsq")
            nc.vector.tensor_mul(wvsq, wvT_sb, wvT_sb)
            ssqv_ps = finps.tile([1, 64], F32, tag="fin", name="ssqv_ps")
            for t in range(2):
                nc.tensor.matmul(ssqv_ps, ones[:, 0:1], wvsq[:, t, :],
                                 start=(t == 0), stop=(t == 1))
            vln = work.tile([1, 64], F32, tag="vln")
            nc.scalar.activation(vln, ssqv_ps, AF.Ln, bias=0.0, scale=1.0)
            vn = work.tile([1, 64], F32, tag="vn")
            nc.scalar.activation(vn, vln, AF.Exp, bias=0.0, scale=0.5)
            vne = work.tile([1, 64], F32, tag="vne")
            nc.vector.tensor_scalar_add(vne, vn, W_EPS)
            svrow = work.tile([1, 64], F32, tag="svrow")
            nc.vector.reciprocal(svrow, vne)
            svbc_ps = finps.tile([128, 64], F32, tag="fin", name="svbc_ps")
            nc.tensor.matmul(svbc_ps, ones[0:1, 0:128], svrow[0:1, :],
                             start=True, stop=True)
            for t in range(2):
                nc.vector.tensor_mul(wvT_sb[:, t, :], wvT_sb[:, t, :], svbc_ps)

            # ---------------- P2: convs ----------------
            for ch in range(NCH):
                sl = slice(ch * 512, ch * 512 + 512)
                pq = finps.tile([128, 512], F32, tag="fin", name=f"pq{ch}")
                pk = finps.tile([128, 512], F32, tag="fin", name=f"pk{ch}")
                for t in range(2):
                    nc.tensor.matmul(pq[0:64, :], wqT_sb[:, t, :],
                                     x_sb[:, t, sl], start=(t == 0), stop=(t == 1))
                for t in range(2):
                    nc.tensor.matmul(pk[0:64, :], wkT_sb[:, t, :],
                                     x_sb[:, t, sl], start=(t == 0), stop=(t == 1))
                nc.vector.tensor_scalar(qT[:, sl], pq[0:64, :], sqq[:, 0:1],
                                        None, op0=ALU.mult)
                nc.vector.tensor_scalar(kT[:, sl], pk[0:64, :], sqk[:, 0:1],
                                        None, op0=ALU.mult)
            for pt in range(KT):
                pv = finps.tile([128, 64], F32, tag="fin", name=f"pv{pt}")
                for t in range(2):
                    nc.tensor.matmul(pv, x_sb[:, t, pt * 128:pt * 128 + 128],
                                     wvT_sb[:, t, :], start=(t == 0), stop=(t == 1))
                nc.vector.tensor_copy(va[:, pt * 66:pt * 66 + 32], pv[:, 0:32])
                nc.vector.tensor_copy(va[:, pt * 66 + 33:pt * 66 + 65], pv[:, 32:64])

            # residual pre-scale of x (x only needed for the final add now)
            nc.vector.tensor_scalar_mul(x_sb.rearrange("p t f -> p (t f)"),
                                        x_sb.rearrange("p t f -> p (t f)"), float(C_X))

            # ---------------- P3: v pixel-norm stats ----------------
            va4 = va.rearrange("p (kt h e) -> p kt h e", kt=KT, h=2)
            vsq = scr.tile([128, KT, 2, HD], F32, tag="vsq")
            nc.vector.tensor_mul(vsq, va4[:, :, :, 0:HD], va4[:, :, :, 0:HD])
            msum = sg.tile([128, 64], F32)
            nc.vector.tensor_reduce(msum, vsq, axis=mybir.AxisListType.X, op=ALU.add)
            lnv = sg.tile([128, 64], F32)
            nc.scalar.activation(lnv, msum, AF.Ln, bias=eps_col[:, 0:1], scale=1.0 / HD)
            nc.vector.tensor_scalar_mul(vbias, lnv, -0.5)
            lnv3 = lnv.rearrange("p (kt h) -> p kt h", h=2)
            va3 = va.rearrange("p (kt x) -> p kt x", kt=KT)
            for h in range(2):
                nc.scalar.activation(va3[:, :, 32 + 33 * h:33 + 33 * h],
                                     lnv3[:, :, h:h + 1], AF.Exp, bias=0.0, scale=0.5)

            # ---------------- P4: k exp scales (column form) ----------------
            ksq = scr.tile([64, HW], F32, tag="qksq", name="ksq")
            nc.vector.tensor_mul(ksq, kT, kT)
            psk = finps.tile([128, 64], F32, tag="fin", name="psk")
            for kt in range(KT):
                for h in range(2):
                    nc.tensor.matmul(psk[:, kt * 2 + h:kt * 2 + h + 1],
                                     ksq[32 * h:32 * h + 32, kt * 128:kt * 128 + 128],
                                     ones[32 * h:32 * h + 32, 0:1],
                                     start=True, stop=True)
            lnk = work.tile([128, 64], F32, tag="lnk")
            nc.scalar.activation(lnk, psk, AF.Ln, bias=eps_col[:, 0:1], scale=1.0 / HD)
            nc.scalar.activation(skcol, lnk, AF.Exp, bias=lniq_col[:, 0:1], scale=-0.5)

            # ---------------- P5: q pixel-norm multiply ----------------
            qsq = scr.tile([64, HW], F32, tag="qksq", name="qsq")
            nc.vector.tensor_mul(qsq, qT, qT)
            psq = finps.tile([128, 64], F32, tag="fin", name="psq")
            for h in range(2):
                for t in range(KT):
                    nc.tensor.matmul(psq[:, h * 32 + t:h * 32 + t + 1],
                                     qsq[32 * h:32 * h + 32, t * 128:t * 128 + 128],
                                     ones[32 * h:32 * h + 32, 0:1],
                                     start=True, stop=True)
            lnq = work.tile([128, 64], F32, tag="lnq")
            nc.scalar.activation(lnq, psq, AF.Ln, bias=eps_col[:, 0:1], scale=1.0 / HD)
            sqc = work.tile([128, 64], F32, tag="sqc")
            nc.scalar.activation(sqc, lnq, AF.Exp, bias=0.0, scale=-0.5)
            for h in range(2):
                nc.sync.dma_start(out=bqs[h][:].rearrange("t p -> p t"),
                                  in_=sqc[:, h * 32:h * 32 + 32])
                nc.sync.dma_start(out=sqrow[0:1, h * HW:(h + 1) * HW],
                                  in_=bqs[h][:].rearrange("t p -> (t p)"))
            for ch in range(NCH):
                sl = slice(ch * 512, ch * 512 + 512)
                bch0 = finps.tile([128, 512], F32, tag="fin", name=f"bcq0_{ch}")
                bch1 = finps.tile([128, 512], F32, tag="fin", name=f"bcq1_{ch}")
                nc.tensor.matmul(bch0[0:32, :], ones[0:1, 0:32],
                                 sqrow[0:1, ch * 512:ch * 512 + 512],
                                 start=True, stop=True, tile_position=(0, 0))
                nc.tensor.matmul(bch1[32:64, :], ones[0:1, 0:32],
                                 sqrow[0:1, HW + ch * 512:HW + ch * 512 + 512],
                                 start=True, stop=True, tile_position=(0, 32))
                nc.vector.tensor_mul(qT[0:32, sl], qT[0:32, sl], bch0[0:32, :])
                nc.vector.tensor_mul(qT[32:64, sl], qT[32:64, sl], bch1[32:64, :])

            # ---------------- P6: attention ----------------
            for pr in range(NCH // 2):
                qcs = (2 * pr, 2 * pr + 1)
                acc = {}
                for h in range(2):
                    for j in range(2):
                        acc[(h, j)] = accps.tile([128, 512], F32, tag="acc",
                                                 name=f"acc{pr}_{h}{j}")
                for kt in range(KT):
                    st, sp = kt == 0, kt == KT - 1
                    for h in range(2):
                        hs = slice(32 * h, 32 * h + 32)
                        sc = scps.tile([128, 1024], F32, tag="sc", name=f"sc{pr}_{kt}_{h}")
                        for j in range(2):
                            nc.tensor.matmul(sc[:, j * 512:j * 512 + 512],
                                             kT[hs, kt * 128:kt * 128 + 128],
                                             qT[hs, qcs[j] * 512:qcs[j] * 512 + 512],
                                             start=True, stop=True)
                        e = ep.tile([128, 1024], F32, tag="e", name=f"e{pr}_{kt}_{h}")
                        nc.scalar.activation(e, sc, AF.Exp,
                                             bias=vbias[:, kt * 2 + h:kt * 2 + h + 1],
                                             scale=skcol[:, kt * 2 + h:kt * 2 + h + 1])
                        lhs = va[:, kt * 66 + 33 * h:kt * 66 + 33 * h + 33]
                        for j in range(2):
                            if h == 0:
                                nc.tensor.matmul(acc[(h, j)][0:33, :], lhs,
                                                 e[:, j * 512:j * 512 + 512],
                                                 start=st, stop=sp, tile_position=(0, 0))
                            else:
                                nc.tensor.matmul(acc[(h, j)][64:97, :], lhs,
                                                 e[:, j * 512:j * 512 + 512],
                                                 start=st, stop=sp, tile_position=(0, 64))
                # finalize both chunks of the pair
                for j in range(2):
                    qc = qcs[j]
                    a0, a1 = acc[(0, j)], acc[(1, j)]
                    rden = work.tile([128, 512], F32, tag="rden", name=f"rden{pr}_{j}")
                    nc.vector.reciprocal(rden[32:33, :], a0[32:33, :])
                    nc.vector.reciprocal(rden[96:97, :], a1[96:97, :])
                    bc2a = finps.tile([128, 512], F32, tag="fin", name=f"bc2a_{pr}_{j}")
                    bc2b = finps.tile([128, 512], F32, tag="fin", name=f"bc2b_{pr}_{j}")
                    nc.tensor.matmul(bc2a[0:32, :], ones[32:33, 0:32],
                                     rden[32:33, :], start=True, stop=True,
                                     tile_position=(32, 0))
                    nc.tensor.matmul(bc2b[64:96, :], ones[96:97, 0:32],
                                     rden[96:97, :], start=True, stop=True,
                                     tile_position=(96, 64))
                    yfin = op.tile([128, 512], F32, tag="yfin", name=f"yfin{pr}_{j}")
                    nc.vector.tensor_copy(yfin[0:32, :], a0[0:32, :])
                    nc.vector.tensor_copy(yfin[64:96, :], a1[64:96, :])
                    nc.vector.tensor_mul(yfin[0:32, :], yfin[0:32, :], bc2a[0:32, :])
                    nc.vector.tensor_mul(yfin[64:96, :], yfin[64:96, :], bc2b[64:96, :])
                    # out conv + residual + store
                    for mt in range(2):
                        opsa = finps.tile([128, 512], F32, tag="fin", name=f"opsa{pr}_{j}{mt}")
                        opsb = finps.tile([128, 512], F32, tag="fin", name=f"opsb{pr}_{j}{mt}")
                        nc.tensor.matmul(opsa, woT4_sb[0:32, mt * 128:mt * 128 + 128],
                                         yfin[0:32, :], start=True, stop=True)
                        nc.tensor.matmul(opsb, woT4_sb[64:96, mt * 128:mt * 128 + 128],
                                         yfin[64:96, :], start=True, stop=True)
                        osb = op.tile([128, 512], F32, tag="osb", name=f"osb{pr}_{j}{mt}")
                        nc.vector.scalar_tensor_tensor(
                            osb, opsa, swo[:, mt:mt + 1],
                            x_sb[:, mt, qc * 512:qc * 512 + 512], ALU.mult, ALU.add)
                        nc.vector.scalar_tensor_tensor(
                            osb, opsb, swo[:, mt:mt + 1], osb, ALU.mult, ALU.add)
                        nc.sync.dma_start(
                            out=y_d[:].rearrange("(t p) f -> p t f", p=128)[:, mt:mt + 1, qc * 512:qc * 512 + 512],
                            in_=osb)

    if split:
        _split_waits(nc)
    return nc


_PROG = None
last_results = None


def kernel(x, w_qkv, w_out, num_heads):
    global _PROG
    x = np.asarray(x, dtype=np.float32)
    W = np.asarray(w_qkv, dtype=np.float32)[:, :, 0, 0]
    WO = np.asarray(w_out, dtype=np.float32)[:, :, 0, 0]
    b_, c_, hh, ww = x.shape
    assert (b_, c_, hh * ww) == (2, C, HW)

    if _PROG is None:
        _PROG = build_program()
    nc = _PROG

    in_maps = []
    for core in range(8):
        b = core // 4
        h0 = 2 * (core % 4)
        h1 = h0 + 1
        rq = np.concatenate([W[h0 * HD:(h0 + 1) * HD], W[h1 * HD:(h1 + 1) * HD]], 0)
        rk = np.concatenate([W[C + h0 * HD:C + (h0 + 1) * HD],
                             W[C + h1 * HD:C + (h1 + 1) * HD]], 0)
        rv = np.concatenate([W[2 * C + h0 * HD:2 * C + (h0 + 1) * HD],
                             W[2 * C + h1 * HD:2 * C + (h1 + 1) * HD]], 0)
        woT4 = np.zeros((128, C), np.float32)
        woT4[0:32] = WO[:, h0 * HD:(h0 + 1) * HD].T
        woT4[64:96] = WO[:, h1 * HD:(h1 + 1) * HD].T
        in_maps.append({
            "x": np.ascontiguousarray(x[b].reshape(C, HW)),
            "wqn": np.ascontiguousarray(rq),
            "wkn": np.ascontiguousarray(rk),
            "wqT": np.ascontiguousarray(rq.T),
            "wkT": np.ascontiguousarray(rk.T),
            "wvT": np.ascontiguousarray(rv.T),
            "won": np.ascontiguousarray(WO),
            "woT4": woT4,
        })

    res = run_bass_kernel_spmd(nc, in_maps, list(range(8)))
    global last_results
    last_results = res
    outs = [r["y"] for r in res.results]
    full = np.zeros((2, C, HW), np.float32)
    for core in range(8):
        full[core // 4] += outs[core]
    return full.reshape(b_, c_, hh, ww)



# revision 4
# speedup vs baseline: 3.3717x; 3.3717x over previous
"""CosineAttention Trainium2 kernel (8 NeuronCores, SPMD).

Sharding: 16 (batch, head) pairs -> 8 cores, 2 heads (one batch) per core.
Per core, attention runs in transposed-score layout (scoresT[kpos, qpos]) so
both attention matmuls contract over the partition dim with no transposes:
  MM1: scoresT = kT.T-slice.T @ qT-slice        (K=32 head_dim)
  exp: ACT Exp with per-partition scale=(k pixel-norm * 1/sqrt(hd)) and
       bias=ln(v pixel-norm) folded in -> e = sv[k]*exp(true scoreT)
  MM2: lhsT = [v_hat | 1/sv] (M=33): accumulates y^T rows and the softmax
       denominator row in one PSUM accumulation group.
Attention matmul operands (q, k, v, e, out-conv) are bf16: 1 col/cycle on the
PE vs 4 for fp32, with fp32 PSUM accumulation. The convs stay fp32 for exact
pre-norm stats. The kt-loop is software-pipelined: MM1(kt) -> exp(kt) ->
MM2(kt-1) so the PE never waits on the ACT exp of the current tile.
Weight-norm scales fold into conv PSUM evictions (per-partition scalars).
The out-conv is computed per-core on the core's 64 attention channels; the
8 partial results (each including 1/4 of the x residual term) are summed on
host per batch -- that sum is the gather/unshard step.
"""
import numpy as np
import concourse.bass as bass
import concourse.tile as tile
from concourse import mybir
from concourse.bass_utils import run_bass_kernel_spmd

F32 = mybir.dt.float32
BF16 = mybir.dt.bfloat16
AF = mybir.ActivationFunctionType
ALU = mybir.AluOpType

EPS = 1e-4
MP_T = 0.3
INV_SCALE = 1.0 / np.sqrt(MP_T ** 2 + (1.0 - MP_T) ** 2)
C = 256          # channels
HW = 4096        # pixels
HD = 32          # head dim
NCH = 8          # 512-wide pixel chunks
KT = 32          # 128-wide kpos tiles
LOG_ISQ_HD = float(np.log(1.0 / np.sqrt(HD)))
C_X = 0.25 * (1.0 - MP_T) * INV_SCALE     # per-core share of residual
C_Y = MP_T * INV_SCALE                    # folded into w_out scale
W_EPS = 16.0 * EPS                        # sqrt(fan_in)*EPS with fan_in=256


def _split_waits(nc):
    """This walrus accepts 1 sync wait per engine instruction: hoist extras
    into preceding NoOps on the same engine (engines are in-order)."""
    for f in nc.m.functions:
        for bb in f.blocks:
            newlist = []
            for inst in bb.instructions:
                si = inst.sync_info
                if si is not None and si.on_wait is not None and len(si.on_wait) > 1:
                    waits = list(si.on_wait)
                    if "DMA" in type(inst).__name__:
                        # keep the compute-engine sem on the DMA descriptor;
                        # hoist DMA-queue sems (monotonic, engine-stall safe)
                        hw = [w for w in waits if str(w.ant_name).startswith("DMA")]
                        eng = [w for w in waits if not str(w.ant_name).startswith("DMA")]
                        if eng:
                            keep, extra = eng[-1:], hw + eng[:-1]
                        else:
                            keep, extra = hw[-1:], hw[:-1]
                    else:
                        extra, keep = waits[:-1], waits[-1:]
                    for idx, w in enumerate(extra):
                        nop = mybir.InstNoOp(
                            name=f"{inst.name}_ws{idx}", ins=[], outs=[],
                            sync_info=mybir.SyncInfo(on_wait=[w], on_update=[]))
                        nop.engine = inst.engine
                        newlist.append(nop)
                    inst.sync_info = mybir.SyncInfo(
                        on_wait=keep, on_update=list(si.on_update or []))
                newlist.append(inst)
            bb.instructions = newlist


def _weight_scale_rows(nc, work, nat_ap, p):
    """Per-row weight-norm scale s = 1/(||w_row|| + 16*eps) for natural-layout
    [p, 256] weight rows. Returns a [p, 1] sbuf AP."""
    sq = work.tile([p, 256], F32, tag="wsq", name=f"wsq_{nc.next_id()}")
    nc.vector.tensor_mul(sq, nat_ap, nat_ap)
    ssq = work.tile([p, 1], F32, tag="wssq", name=f"wssq_{nc.next_id()}")
    nc.vector.tensor_reduce(ssq, sq, axis=mybir.AxisListType.X, op=ALU.add)
    ln = work.tile([p, 1], F32, tag="wln", name=f"wln_{nc.next_id()}")
    nc.scalar.activation(ln, ssq, AF.Ln, bias=0.0, scale=1.0)
    n = work.tile([p, 1], F32, tag="wn", name=f"wn_{nc.next_id()}")
    nc.scalar.activation(n, ln, AF.Exp, bias=0.0, scale=0.5)
    ne = work.tile([p, 1], F32, tag="wne", name=f"wne_{nc.next_id()}")
    nc.vector.tensor_scalar_add(ne, n, W_EPS)
    s = work.tile([p, 1], F32, tag="ws", name=f"ws_{nc.next_id()}")
    nc.vector.reciprocal(s, ne)
    return s


def build_program(split=True):
    nc = bass.Bass()
    x_d = nc.declare_dram_parameter("x", [C, HW], F32, isOutput=False)
    wqn_d = nc.declare_dram_parameter("wqn", [64, C], F32, isOutput=False)
    wkn_d = nc.declare_dram_parameter("wkn", [64, C], F32, isOutput=False)
    wqT_d = nc.declare_dram_parameter("wqT", [C, 64], F32, isOutput=False)
    wkT_d = nc.declare_dram_parameter("wkT", [C, 64], F32, isOutput=False)
    wvT_d = nc.declare_dram_parameter("wvT", [C, 64], F32, isOutput=False)
    won_d = nc.declare_dram_parameter("won", [C, C], F32, isOutput=False)
    woT4_d = nc.declare_dram_parameter("woT4", [128, C], F32, isOutput=False)
    y_d = nc.declare_dram_parameter("y", [C, HW], F32, isOutput=True)
    bq0_d = nc.dram_tensor("bq0", [32, 128], BF16)
    bq1_d = nc.dram_tensor("bq1", [32, 128], BF16)
    bqs = [bq0_d, bq1_d]

    with tile.TileContext(nc) as tc:
        with tc.tile_pool(name="singles", bufs=1) as sg, \
             tc.tile_pool(name="work", bufs=2) as work, \
             tc.tile_pool(name="scratch", bufs=2) as scr, \
             tc.tile_pool(name="epool", bufs=4) as ep, \
             tc.tile_pool(name="opool", bufs=4) as op, \
             tc.tile_pool(name="scps", bufs=2, space="PSUM") as scps, \
             tc.tile_pool(name="accps", bufs=4, space="PSUM") as accps:

            # ---------------- P0: loads ----------------
            x_sb = sg.tile([128, 2, HW], F32)
            nc.sync.dma_start(out=x_sb, in_=x_d[:].rearrange("(t p) f -> p t f", p=128))
            wqT_sb = sg.tile([128, 2, 64], F32)
            nc.sync.dma_start(out=wqT_sb, in_=wqT_d[:].rearrange("(t p) m -> p t m", p=128))
            wkT_sb = sg.tile([128, 2, 64], F32)
            nc.sync.dma_start(out=wkT_sb, in_=wkT_d[:].rearrange("(t p) m -> p t m", p=128))
            wvT_sb = sg.tile([128, 2, 64], F32)
            nc.sync.dma_start(out=wvT_sb, in_=wvT_d[:].rearrange("(t p) m -> p t m", p=128))
            wqn_sb = sg.tile([64, C], F32)
            nc.sync.dma_start(out=wqn_sb, in_=wqn_d[:])
            wkn_sb = sg.tile([64, C], F32)
            nc.sync.dma_start(out=wkn_sb, in_=wkn_d[:])
            won_sb = sg.tile([128, 2, C], F32)
            nc.sync.dma_start(out=won_sb, in_=won_d[:].rearrange("(t p) m -> p t m", p=128))
            woT4_sb = sg.tile([128, C], F32)
            nc.sync.dma_start(out=woT4_sb, in_=woT4_d[:])
            ones = sg.tile([128, 128], F32)
            nc.vector.memset(ones, 1.0)
            ones_bf = sg.tile([128, 64], BF16)
            nc.vector.memset(ones_bf, 1.0)
            eps_col = sg.tile([128, 1], F32)
            nc.vector.memset(eps_col, EPS)
            lniq_col = sg.tile([128, 1], F32)
            nc.vector.memset(lniq_col, LOG_ISQ_HD)

            woT4_bf = sg.tile([128, C], BF16)
            nc.vector.tensor_copy(woT4_bf, woT4_sb)

            qT = sg.tile([64, HW], BF16)
            kT = sg.tile([64, HW], BF16)
            va = sg.tile([128, KT * 66], BF16)  # per kt: v_h0(32)|sinv_h0|v_h1(32)|sinv_h1
            vbias = sg.tile([128, 64], F32)     # ln(sv), col = kt*2 + h
            skcol = sg.tile([128, 64], F32)     # exp scale, col = kt*2 + h
            sqrow = sg.tile([1, 2 * HW], BF16)  # q norm scales row form per head

            # ---------------- P1: weight-norm scales ----------------
            sqq = _weight_scale_rows(nc, work, wqn_sb[:, :], 64)
            sqk = _weight_scale_rows(nc, work, wkn_sb[:, :], 64)
            swo = sg.tile([128, 2], F32)
            wosq = work.tile([128, 2, C], F32, tag="wosq")
            nc.vector.tensor_mul(wosq, won_sb, won_sb)
            wossq = work.tile([128, 2], F32, tag="wossq")
            nc.vector.tensor_reduce(wossq, wosq, axis=mybir.AxisListType.X, op=ALU.add)
            woln = work.tile([128, 2], F32, tag="woln")
            nc.scalar.activation(woln, wossq, AF.Ln, bias=0.0, scale=1.0)
            won_n = work.tile([128, 2], F32, tag="won_n")
            nc.scalar.activation(won_n, woln, AF.Exp, bias=0.0, scale=0.5)
            won_ne = work.tile([128, 2], F32, tag="won_ne")
            nc.vector.tensor_scalar_add(won_ne, won_n, W_EPS)
            swo_inv = work.tile([128, 2], F32, tag="swo_inv")
            nc.vector.reciprocal(swo_inv, won_ne)
            nc.vector.tensor_scalar_mul(swo, swo_inv, float(C_Y))

            # wv column scales: s_v[col] = 1/(||w_v[col]|| + 16 eps), fold into wvT
            wvsq = work.tile([128, 2, 64], F32, tag="wvsq")
            nc.vector.tensor_mul(wvsq, wvT_sb, wvT_sb)
            ssqv_ps = accps.tile([1, 64], F32, tag="acc", name="ssqv_ps")
            for t in range(2):
                nc.tensor.matmul(ssqv_ps, ones[:, 0:1], wvsq[:, t, :],
                                 start=(t == 0), stop=(t == 1))
            vln = work.tile([1, 64], F32, tag="vln")
            nc.scalar.activation(vln, ssqv_ps, AF.Ln, bias=0.0, scale=1.0)
            vn = work.tile([1, 64], F32, tag="vn")
            nc.scalar.activation(vn, vln, AF.Exp, bias=0.0, scale=0.5)
            vne = work.tile([1, 64], F32, tag="vne")
            nc.vector.tensor_scalar_add(vne, vn, W_EPS)
            svrow = work.tile([1, 64], F32, tag="svrow")
            nc.vector.reciprocal(svrow, vne)
            svbc_ps = accps.tile([128, 64], F32, tag="acc", name="svbc_ps")
            nc.tensor.matmul(svbc_ps, ones[0:1, 0:128], svrow[0:1, :],
                             start=True, stop=True)
            for t in range(2):
                nc.vector.tensor_mul(wvT_sb[:, t, :], wvT_sb[:, t, :], svbc_ps)

            # ---------------- P2: convs (fp32, exact pre-norm stats) --------
            for ch in range(NCH):
                sl = slice(ch * 512, ch * 512 + 512)
                pq = accps.tile([128, 512], F32, tag="acc", name=f"pq{ch}")
                pk = accps.tile([128, 512], F32, tag="acc", name=f"pk{ch}")
                for t in range(2):
                    nc.tensor.matmul(pq[0:64, :], wqT_sb[:, t, :],
                                     x_sb[:, t, sl], start=(t == 0), stop=(t == 1))
                for t in range(2):
                    nc.tensor.matmul(pk[0:64, :], wkT_sb[:, t, :],
                                     x_sb[:, t, sl], start=(t == 0), stop=(t == 1))
                nc.vector.tensor_scalar(qT[:, sl], pq[0:64, :], sqq[:, 0:1],
                                        None, op0=ALU.mult)
                nc.vector.tensor_scalar(kT[:, sl], pk[0:64, :], sqk[:, 0:1],
                                        None, op0=ALU.mult)
            for pt in range(KT):
                pv = accps.tile([128, 64], F32, tag="acc", name=f"pv{pt}")
                for t in range(2):
                    nc.tensor.matmul(pv, x_sb[:, t, pt * 128:pt * 128 + 128],
                                     wvT_sb[:, t, :], start=(t == 0), stop=(t == 1))
                nc.vector.tensor_copy(va[:, pt * 66:pt * 66 + 32], pv[:, 0:32])
                nc.vector.tensor_copy(va[:, pt * 66 + 33:pt * 66 + 65], pv[:, 32:64])

            # residual pre-scale of x (x only needed for the final add now)
            nc.vector.tensor_scalar_mul(x_sb.rearrange("p t f -> p (t f)"),
                                        x_sb.rearrange("p t f -> p (t f)"), float(C_X))

            # ---------------- P3: v pixel-norm stats ----------------
            # squares from the exact fp32 psum copies would be nicer, but the
            # bf16 v is what MM2 consumes; bf16 norm error ~0.4% is fine.
            va4 = va.rearrange("p (kt h e) -> p kt h e", kt=KT, h=2)
            vsq = scr.tile([128, KT, 2, HD], F32, tag="vsq")
            nc.vector.tensor_mul(vsq, va4[:, :, :, 0:HD], va4[:, :, :, 0:HD])
            msum = sg.tile([128, 64], F32)
            nc.vector.tensor_reduce(msum, vsq, axis=mybir.AxisListType.X, op=ALU.add)
            lnv = sg.tile([128, 64], F32)
            nc.scalar.activation(lnv, msum, AF.Ln, bias=eps_col[:, 0:1], scale=1.0 / HD)
            nc.vector.tensor_scalar_mul(vbias, lnv, -0.5)
            lnv3 = lnv.rearrange("p (kt h) -> p kt h", h=2)
            va3 = va.rearrange("p (kt x) -> p kt x", kt=KT)
            for h in range(2):
                nc.scalar.activation(va3[:, :, 32 + 33 * h:33 + 33 * h],
                                     lnv3[:, :, h:h + 1], AF.Exp, bias=0.0, scale=0.5)

            # ---------------- P4: k exp scales (column form) ----------------
            ksq = scr.tile([64, HW], BF16, tag="qksq", name="ksq")
            nc.vector.tensor_mul(ksq, kT, kT)
            psk = accps.tile([128, 64], F32, tag="acc", name="psk")
            for kt in range(KT):
                for h in range(2):
                    nc.tensor.matmul(psk[:, kt * 2 + h:kt * 2 + h + 1],
                                     ksq[32 * h:32 * h + 32, kt * 128:kt * 128 + 128],
                                     ones_bf[32 * h:32 * h + 32, 0:1],
                                     start=True, stop=True)
            lnk = work.tile([128, 64], F32, tag="lnk")
            nc.scalar.activation(lnk, psk, AF.Ln, bias=eps_col[:, 0:1], scale=1.0 / HD)
            nc.scalar.activation(skcol, lnk, AF.Exp, bias=lniq_col[:, 0:1], scale=-0.5)

            # ---------------- P5: q pixel-norm multiply ----------------
            qsq = scr.tile([64, HW], BF16, tag="qksq", name="qsq")
            nc.vector.tensor_mul(qsq, qT, qT)
            psq = accps.tile([128, 64], F32, tag="acc", name="psq")
            for h in range(2):
                for t in range(KT):
                    nc.tensor.matmul(psq[:, h * 32 + t:h * 32 + t + 1],
                                     qsq[32 * h:32 * h + 32, t * 128:t * 128 + 128],
                                     ones_bf[32 * h:32 * h + 32, 0:1],
                                     start=True, stop=True)
            lnq = work.tile([128, 64], F32, tag="lnq")
            nc.scalar.activation(lnq, psq, AF.Ln, bias=eps_col[:, 0:1], scale=1.0 / HD)
            sqc = work.tile([128, 64], BF16, tag="sqc")
            nc.scalar.activation(sqc, lnq, AF.Exp, bias=0.0, scale=-0.5)
            for h in range(2):
                nc.sync.dma_start(out=bqs[h][:].rearrange("t p -> p t"),
                                  in_=sqc[:, h * 32:h * 32 + 32])
                nc.sync.dma_start(out=sqrow[0:1, h * HW:(h + 1) * HW],
                                  in_=bqs[h][:].rearrange("t p -> (t p)"))
            for ch in range(NCH):
                sl = slice(ch * 512, ch * 512 + 512)
                bch0 = accps.tile([128, 512], F32, tag="acc", name=f"bcq0_{ch}")
                bch1 = accps.tile([128, 512], F32, tag="acc", name=f"bcq1_{ch}")
                nc.tensor.matmul(bch0[0:32, :], ones_bf[0:1, 0:32],
                                 sqrow[0:1, ch * 512:ch * 512 + 512],
                                 start=True, stop=True, tile_position=(0, 0))
                nc.tensor.matmul(bch1[32:64, :], ones_bf[0:1, 0:32],
                                 sqrow[0:1, HW + ch * 512:HW + ch * 512 + 512],
                                 start=True, stop=True, tile_position=(0, 32))
                nc.vector.tensor_mul(qT[0:32, sl], qT[0:32, sl], bch0[0:32, :])
                nc.vector.tensor_mul(qT[32:64, sl], qT[32:64, sl], bch1[32:64, :])

            # ---------------- P6: attention ----------------
            for pr in range(NCH // 2):
                qcs = (2 * pr, 2 * pr + 1)
                acc = {}
                for j in range(2):
                    for h in range(2):
                        acc[(h, j)] = accps.tile([128, 512], F32, tag="acc",
                                                 name=f"acc{pr}_{h}{j}")
                sc = {}
                e = {}
                # software-pipelined kt loop: MM1(kt) || exp(kt) || MM2(kt-1)
                for kt in range(KT + 1):
                    if kt < KT:
                        for h in range(2):
                            sc[(kt, h)] = scps.tile([128, 1024], F32, tag="sc",
                                                    name=f"sc{pr}_{kt}_{h}")
                        for j in range(2):
                            for h in range(2):   # h fastest: LDW row-groups alternate
                                nc.tensor.matmul(
                                    sc[(kt, h)][:, j * 512:j * 512 + 512],
                                    kT[32 * h:32 * h + 32, kt * 128:kt * 128 + 128],
                                    qT[32 * h:32 * h + 32,
                                       qcs[j] * 512:qcs[j] * 512 + 512],
                                    start=True, stop=True)
                        for h in range(2):
                            eh = ep.tile([128, 1024], BF16, tag="e",
                                         name=f"e{pr}_{kt}_{h}")
                            nc.scalar.activation(
                                eh, sc[(kt, h)], AF.Exp,
                                bias=vbias[:, kt * 2 + h:kt * 2 + h + 1],
                                scale=skcol[:, kt * 2 + h:kt * 2 + h + 1])
                            e[(kt, h)] = eh
                    if kt >= 1:
                        kp = kt - 1
                        st, sp = kp == 0, kp == KT - 1
                        for h in range(2):
                            lhs = va[:, kp * 66 + 33 * h:kp * 66 + 33 * h + 33]
                            for j in range(2):
                                if h == 0:
                                    nc.tensor.matmul(
                                        acc[(h, j)][0:33, :], lhs,
                                        e[(kp, h)][:, j * 512:j * 512 + 512],
                                        start=st, stop=sp, tile_position=(0, 0))
                                else:
                                    nc.tensor.matmul(
                                        acc[(h, j)][64:97, :], lhs,
                                        e[(kp, h)][:, j * 512:j * 512 + 512],
                                        start=st, stop=sp, tile_position=(0, 64))
                        e.pop((kp, 0)), e.pop((kp, 1))
                # finalize: first drain all 4 acc banks to SBUF (frees the
                # 'acc' ring for the bc2/out-conv tiles and the next pr)
                rden = {}
                yfin = {}
                for j in range(2):
                    a0, a1 = acc[(0, j)], acc[(1, j)]
                    rd = work.tile([128, 512], BF16, tag="rden", name=f"rden{pr}_{j}")
                    with nc.allow_low_precision(reason="softmax denom recip to bf16"):
                        nc.vector.reciprocal(rd[32:33, :], a0[32:33, :])
                        nc.vector.reciprocal(rd[96:97, :], a1[96:97, :])
                    yf = op.tile([128, 512], F32, tag="yfin", name=f"yfin{pr}_{j}")
                    nc.vector.tensor_copy(yf[0:32, :], a0[0:32, :])
                    nc.vector.tensor_copy(yf[64:96, :], a1[64:96, :])
                    rden[j], yfin[j] = rd, yf
                for j in range(2):
                    qc = qcs[j]
                    rd, yf = rden[j], yfin[j]
                    yfb = op.tile([128, 512], BF16, tag="yfb", name=f"yfb{pr}_{j}")
                    bc2a = accps.tile([128, 512], F32, tag="acc", name=f"bc2a_{pr}_{j}")
                    bc2b = accps.tile([128, 512], F32, tag="acc", name=f"bc2b_{pr}_{j}")
                    nc.tensor.matmul(bc2a[0:32, :], ones_bf[32:33, 0:32],
                                     rd[32:33, :], start=True, stop=True,
                                     tile_position=(32, 0))
                    nc.tensor.matmul(bc2b[64:96, :], ones_bf[96:97, 0:32],
                                     rd[96:97, :], start=True, stop=True,
                                     tile_position=(96, 64))
                    nc.vector.tensor_mul(yfb[0:32, :], yf[0:32, :], bc2a[0:32, :])
                    nc.vector.tensor_mul(yfb[64:96, :], yf[64:96, :], bc2b[64:96, :])
                    # out conv + residual + store
                    for mt in range(2):
                        opsa = accps.tile([128, 512], F32, tag="acc",
                                          name=f"opsa{pr}_{j}{mt}")
                        opsb = accps.tile([128, 512], F32, tag="acc",
                                          name=f"opsb{pr}_{j}{mt}")
                        nc.tensor.matmul(opsa, woT4_bf[0:32, mt * 128:mt * 128 + 128],
                                         yfb[0:32, :], start=True, stop=True)
                        nc.tensor.matmul(opsb, woT4_bf[64:96, mt * 128:mt * 128 + 128],
                                         yfb[64:96, :], start=True, stop=True)
                        osb = op.tile([128, 512], F32, tag="osb", name=f"osb{pr}_{j}{mt}")
                        nc.vector.scalar_tensor_tensor(
                            osb, opsa, swo[:, mt:mt + 1],
                            x_sb[:, mt, qc * 512:qc * 512 + 512], ALU.mult, ALU.add)
                        nc.vector.scalar_tensor_tensor(
                            osb, opsb, swo[:, mt:mt + 1], osb, ALU.mult, ALU.add)
                        nc.sync.dma_start(
                            out=y_d[:].rearrange("(t p) f -> p t f", p=128)[:, mt:mt + 1, qc * 512:qc * 512 + 512],
                            in_=osb)

    if split:
        _split_waits(nc)
    return nc


_PROG = None
last_results = None


def kernel(x, w_qkv, w_out, num_heads):
    global _PROG
    x = np.asarray(x, dtype=np.float32)
    W = np.asarray(w_qkv, dtype=np.float32)[:, :, 0, 0]
    WO = np.asarray(w_out, dtype=np.float32)[:, :, 0, 0]
    b_, c_, hh, ww = x.shape
    assert (b_, c_, hh * ww) == (2, C, HW)

    if _PROG is None:
        _PROG = build_program()
    nc = _PROG

    in_maps = []
    for core in range(8):
        b = core // 4
        h0 = 2 * (core % 4)
        h1 = h0 + 1
        rq = np.concatenate([W[h0 * HD:(h0 + 1) * HD], W[h1 * HD:(h1 + 1) * HD]], 0)
        rk = np.concatenate([W[C + h0 * HD:C + (h0 + 1) * HD],
                             W[C + h1 * HD:C + (h1 + 1) * HD]], 0)
        rv = np.concatenate([W[2 * C + h0 * HD:2 * C + (h0 + 1) * HD],
                             W[2 * C + h1 * HD:2 * C + (h1 + 1) * HD]], 0)
        woT4 = np.zeros((128, C), np.float32)
        woT4[0:32] = WO[:, h0 * HD:(h0 + 1) * HD].T
        woT4[64:96] = WO[:, h1 * HD:(h1 + 1) * HD].T
        in_maps.append({
            "x": np.ascontiguousarray(x[b].reshape(C, HW)),
            "wqn": np.ascontiguousarray(rq),
            "wkn": np.ascontiguousarray(rk),
            "wqT": np.ascontiguousarray(rq.T),
            "wkT": np.ascontiguousarray(rk.T),
            "wvT": np.ascontiguousarray(rv.T),
            "won": np.ascontiguousarray(WO),
            "woT4": woT4,
        })

    res = run_bass_kernel_spmd(nc, in_maps, list(range(8)))
    global last_results
    last_results = res
    outs = [r["y"] for r in res.results]
    full = np.zeros((2, C, HW), np.float32)
    for core in range(8):
        full[core // 4] += outs[core]
    return full.reshape(b_, c_, hh, ww)


# revision 5
# speedup vs baseline: 3.4276x; 1.0166x over previous
"""CosineAttention Trainium2 kernel (8 NeuronCores, SPMD).

Sharding: 16 (batch, head) pairs -> 8 cores, 2 heads (one batch) per core.
Per core, attention runs in transposed-score layout (scoresT[kpos, qpos]) so
both attention matmuls contract over the partition dim with no transposes:
  MM1: scoresT = kT.T-slice.T @ qT-slice        (K=32 head_dim)
  exp: e = sv[k]*exp(true scoreT), per-partition scale=(k pixel-norm /
       sqrt(hd)) and bias=ln(sv) folded in. Split ~8/13 ACT (exact Exp) and
       ~5/13 DVE (Schraudolph int16 bit-trick producing bf16), so neither
       engine is the bottleneck.
  MM2: lhsT = [v_hat | 1/sv] (M=33): accumulates y^T rows and the softmax
       denominator row in one PSUM accumulation group. h0 and h1 share one
       PSUM bank per q-chunk (rows 0:33 / 64:97).
Attention matmul operands are bf16 (1 col/cycle on the PE vs 4 for fp32).
The kt-loop is software-pipelined (MM1(kt) | exp(kt) | MM2(kt-1)) with a
3-deep score ring, and each pr's finalize is spread into the next pr's kt
slots so the PE queue never blocks on the DVE finalize chain.
The out-conv is computed per-core on the core's 64 attention channels; the
8 partial results (each including 1/4 of the x residual term) are summed on
host per batch -- that sum is the gather/unshard step.
"""
import numpy as np
import concourse.bass as bass
import concourse.tile as tile
from concourse import mybir
from concourse.bass_utils import run_bass_kernel_spmd

F32 = mybir.dt.float32
BF16 = mybir.dt.bfloat16
I16 = mybir.dt.int16
AF = mybir.ActivationFunctionType
ALU = mybir.AluOpType

EPS = 1e-4
MP_T = 0.3
INV_SCALE = 1.0 / np.sqrt(MP_T ** 2 + (1.0 - MP_T) ** 2)
C = 256          # channels
HW = 4096        # pixels
HD = 32          # head dim
NCH = 8          # 512-wide pixel chunks
KT = 32          # 128-wide kpos tiles
LOG_ISQ_HD = float(np.log(1.0 / np.sqrt(HD)))
C_X = 0.25 * (1.0 - MP_T) * INV_SCALE     # per-core share of residual
C_Y = MP_T * INV_SCALE                    # folded into w_out scale
W_EPS = 16.0 * EPS                        # sqrt(fan_in)*EPS with fan_in=256
S16 = float(2 ** 7 / np.log(2.0))         # Schraudolph bf16 scale
B16 = float(127 * 2 ** 7 - 7.42)          # Schraudolph bf16 bias (min max-err)
ACT_NUM, ACT_DEN = 8, 13                  # ~61.5% of exp units on ACT


def _split_waits(nc):
    """This walrus accepts 1 sync wait per engine instruction: hoist extras
    into preceding NoOps on the same engine (engines are in-order)."""
    for f in nc.m.functions:
        for bb in f.blocks:
            newlist = []
            for inst in bb.instructions:
                si = inst.sync_info
                if si is not None and si.on_wait is not None and len(si.on_wait) > 1:
                    waits = list(si.on_wait)
                    if "DMA" in type(inst).__name__:
                        # keep the compute-engine sem on the DMA descriptor;
                        # hoist DMA-queue sems (monotonic, engine-stall safe)
                        hw = [w for w in waits if str(w.ant_name).startswith("DMA")]
                        eng = [w for w in waits if not str(w.ant_name).startswith("DMA")]
                        if eng:
                            keep, extra = eng[-1:], hw + eng[:-1]
                        else:
                            keep, extra = hw[-1:], hw[:-1]
                    else:
                        extra, keep = waits[:-1], waits[-1:]
                    for idx, w in enumerate(extra):
                        nop = mybir.InstNoOp(
                            name=f"{inst.name}_ws{idx}", ins=[], outs=[],
                            sync_info=mybir.SyncInfo(on_wait=[w], on_update=[]))
                        nop.engine = inst.engine
                        newlist.append(nop)
                    inst.sync_info = mybir.SyncInfo(
                        on_wait=keep, on_update=list(si.on_update or []))
                newlist.append(inst)
            bb.instructions = newlist


def _weight_scale_rows(nc, work, nat_ap, p):
    """Per-row weight-norm scale s = 1/(||w_row|| + 16*eps) for natural-layout
    [p, 256] weight rows. Returns a [p, 1] sbuf AP."""
    sq = work.tile([p, 256], F32, tag="wsq", name=f"wsq_{nc.next_id()}")
    nc.vector.tensor_mul(sq, nat_ap, nat_ap)
    ssq = work.tile([p, 1], F32, tag="wssq", name=f"wssq_{nc.next_id()}")
    nc.vector.tensor_reduce(ssq, sq, axis=mybir.AxisListType.X, op=ALU.add)
    ln = work.tile([p, 1], F32, tag="wln", name=f"wln_{nc.next_id()}")
    nc.scalar.activation(ln, ssq, AF.Ln, bias=0.0, scale=1.0)
    n = work.tile([p, 1], F32, tag="wn", name=f"wn_{nc.next_id()}")
    nc.scalar.activation(n, ln, AF.Exp, bias=0.0, scale=0.5)
    ne = work.tile([p, 1], F32, tag="wne", name=f"wne_{nc.next_id()}")
    nc.vector.tensor_scalar_add(ne, n, W_EPS)
    s = work.tile([p, 1], F32, tag="ws", name=f"ws_{nc.next_id()}")
    nc.vector.reciprocal(s, ne)
    return s


def build_program(split=True):
    nc = bass.Bass()
    x_d = nc.declare_dram_parameter("x", [C, HW], F32, isOutput=False)
    wqn_d = nc.declare_dram_parameter("wqn", [64, C], F32, isOutput=False)
    wkn_d = nc.declare_dram_parameter("wkn", [64, C], F32, isOutput=False)
    wqT_d = nc.declare_dram_parameter("wqT", [C, 64], F32, isOutput=False)
    wkT_d = nc.declare_dram_parameter("wkT", [C, 64], F32, isOutput=False)
    wvT_d = nc.declare_dram_parameter("wvT", [C, 64], F32, isOutput=False)
    won_d = nc.declare_dram_parameter("won", [C, C], F32, isOutput=False)
    woT4_d = nc.declare_dram_parameter("woT4", [128, C], F32, isOutput=False)
    y_d = nc.declare_dram_parameter("y", [C, HW], F32, isOutput=True)

    with tile.TileContext(nc) as tc:
        with tc.tile_pool(name="singles", bufs=1) as sg, \
             tc.tile_pool(name="work", bufs=2) as work, \
             tc.tile_pool(name="scratch", bufs=2) as scr, \
             tc.tile_pool(name="epool", bufs=4) as ep, \
             tc.tile_pool(name="opool", bufs=4) as op, \
             tc.tile_pool(name="scps", bufs=3, space="PSUM") as scps, \
             tc.tile_pool(name="accps", bufs=2, space="PSUM") as accps:

            # ---------------- P0: loads ----------------
            x_sb = sg.tile([128, 2, HW], F32)
            xr = x_d[:].rearrange("(t p) f -> p t f", p=128)
            for blk in range(4):
                fs = slice(blk * 1024, blk * 1024 + 1024)
                nc.sync.dma_start(out=x_sb[:, :, fs], in_=xr[:, :, fs])
            wqT_sb = sg.tile([128, 2, 64], F32)
            nc.sync.dma_start(out=wqT_sb, in_=wqT_d[:].rearrange("(t p) m -> p t m", p=128))
            wkT_sb = sg.tile([128, 2, 64], F32)
            nc.sync.dma_start(out=wkT_sb, in_=wkT_d[:].rearrange("(t p) m -> p t m", p=128))
            wvT_sb = sg.tile([128, 2, 64], F32)
            nc.sync.dma_start(out=wvT_sb, in_=wvT_d[:].rearrange("(t p) m -> p t m", p=128))
            wqn_sb = sg.tile([64, C], F32)
            nc.sync.dma_start(out=wqn_sb, in_=wqn_d[:])
            wkn_sb = sg.tile([64, C], F32)
            nc.sync.dma_start(out=wkn_sb, in_=wkn_d[:])
            won_sb = sg.tile([128, 2, C], F32)
            nc.sync.dma_start(out=won_sb, in_=won_d[:].rearrange("(t p) m -> p t m", p=128))
            woT4_sb = sg.tile([128, C], F32)
            nc.sync.dma_start(out=woT4_sb, in_=woT4_d[:])
            ones = sg.tile([128, 128], F32)
            nc.vector.memset(ones, 1.0)
            ones_bf = sg.tile([128, 64], BF16)
            nc.vector.memset(ones_bf, 1.0)
            eps_col = sg.tile([128, 1], F32)
            nc.vector.memset(eps_col, EPS)
            lniq_col = sg.tile([128, 1], F32)
            nc.vector.memset(lniq_col, LOG_ISQ_HD)

            woT4_bf = sg.tile([128, C], BF16)
            nc.vector.tensor_copy(woT4_bf, woT4_sb)

            qT = sg.tile([64, HW], BF16)
            kT = sg.tile([64, HW], BF16)
            va = sg.tile([128, KT * 66], BF16)  # per kt: v_h0(32)|sinv_h0|v_h1(32)|sinv_h1
            vbias = sg.tile([128, 64], F32)     # ln(sv), col = kt*2 + h
            skcol = sg.tile([128, 64], F32)     # exp scale, col = kt*2 + h
            skS16 = sg.tile([128, 64], F32)     # Schraudolph scale = skcol * S16
            vbB16 = sg.tile([128, 64], F32)     # Schraudolph bias = vbias*S16 + B16

            # ---------------- P1: weight-norm scales ----------------
            sqq = _weight_scale_rows(nc, work, wqn_sb[:, :], 64)
            sqk = _weight_scale_rows(nc, work, wkn_sb[:, :], 64)
            swo = sg.tile([128, 2], F32)
            wosq = work.tile([128, 2, C], F32, tag="wosq")
            nc.vector.tensor_mul(wosq, won_sb, won_sb)
            wossq = work.tile([128, 2], F32, tag="wossq")
            nc.vector.tensor_reduce(wossq, wosq, axis=mybir.AxisListType.X, op=ALU.add)
            woln = work.tile([128, 2], F32, tag="woln")
            nc.scalar.activation(woln, wossq, AF.Ln, bias=0.0, scale=1.0)
            won_n = work.tile([128, 2], F32, tag="won_n")
            nc.scalar.activation(won_n, woln, AF.Exp, bias=0.0, scale=0.5)
            won_ne = work.tile([128, 2], F32, tag="won_ne")
            nc.vector.tensor_scalar_add(won_ne, won_n, W_EPS)
            swo_inv = work.tile([128, 2], F32, tag="swo_inv")
            nc.vector.reciprocal(swo_inv, won_ne)
            nc.vector.tensor_scalar_mul(swo, swo_inv, float(C_Y))

            # wv column scales: s_v[col] = 1/(||w_v[col]|| + 16 eps), fold into wvT
            wvsq = work.tile([128, 2, 64], F32, tag="wvsq")
            nc.vector.tensor_mul(wvsq, wvT_sb, wvT_sb)
            ssqv_ps = scps.tile([1, 64], F32, tag="sc", name="ssqv_ps")
            for t in range(2):
                nc.tensor.matmul(ssqv_ps, ones[:, 0:1], wvsq[:, t, :],
                                 start=(t == 0), stop=(t == 1))
            vln = work.tile([1, 64], F32, tag="vln")
            nc.scalar.activation(vln, ssqv_ps, AF.Ln, bias=0.0, scale=1.0)
            vn = work.tile([1, 64], F32, tag="vn")
            nc.scalar.activation(vn, vln, AF.Exp, bias=0.0, scale=0.5)
            vne = work.tile([1, 64], F32, tag="vne")
            nc.vector.tensor_scalar_add(vne, vn, W_EPS)
            svrow = work.tile([1, 64], F32, tag="svrow")
            nc.vector.reciprocal(svrow, vne)
            svbc_ps = scps.tile([128, 64], F32, tag="sc", name="svbc_ps")
            nc.tensor.matmul(svbc_ps, ones[0:1, 0:128], svrow[0:1, :],
                             start=True, stop=True)
            for t in range(2):
                nc.vector.tensor_mul(wvT_sb[:, t, :], wvT_sb[:, t, :], svbc_ps)

            # ---------------- P2: convs (fp32, exact pre-norm stats) --------
            for ch in range(NCH):
                sl = slice(ch * 512, ch * 512 + 512)
                pq = scps.tile([128, 512], F32, tag="sc", name=f"pq{ch}")
                pk = scps.tile([128, 512], F32, tag="sc", name=f"pk{ch}")
                for t in range(2):
                    nc.tensor.matmul(pq[0:64, :], wqT_sb[:, t, :],
                                     x_sb[:, t, sl], start=(t == 0), stop=(t == 1))
                for t in range(2):
                    nc.tensor.matmul(pk[0:64, :], wkT_sb[:, t, :],
                                     x_sb[:, t, sl], start=(t == 0), stop=(t == 1))
                nc.vector.tensor_scalar(qT[:, sl], pq[0:64, :], sqq[:, 0:1],
                                        None, op0=ALU.mult)
                nc.vector.tensor_scalar(kT[:, sl], pk[0:64, :], sqk[:, 0:1],
                                        None, op0=ALU.mult)
            for pt in range(KT):
                pv = scps.tile([128, 64], F32, tag="sc", name=f"pv{pt}")
                for t in range(2):
                    nc.tensor.matmul(pv, x_sb[:, t, pt * 128:pt * 128 + 128],
                                     wvT_sb[:, t, :], start=(t == 0), stop=(t == 1))
                nc.vector.tensor_copy(va[:, pt * 66:pt * 66 + 32], pv[:, 0:32])
                nc.vector.tensor_copy(va[:, pt * 66 + 33:pt * 66 + 65], pv[:, 32:64])

            # residual pre-scale of x (x only needed for the final add now)
            nc.gpsimd.tensor_scalar_mul(x_sb.rearrange("p t f -> p (t f)"),
                                        x_sb.rearrange("p t f -> p (t f)"), float(C_X))

            # ---------------- P3: v pixel-norm stats ----------------
            va4 = va.rearrange("p (kt h e) -> p kt h e", kt=KT, h=2)
            vsq = scr.tile([128, KT, 2, HD], F32, tag="vsq")
            nc.gpsimd.tensor_mul(vsq, va4[:, :, :, 0:HD], va4[:, :, :, 0:HD])
            msum = sg.tile([128, 64], F32)
            nc.vector.tensor_reduce(msum, vsq, axis=mybir.AxisListType.X, op=ALU.add)
            lnv = sg.tile([128, 64], F32)
            nc.scalar.activation(lnv, msum, AF.Ln, bias=eps_col[:, 0:1], scale=1.0 / HD)
            nc.vector.tensor_scalar_mul(vbias, lnv, -0.5)
            lnv3 = lnv.rearrange("p (kt h) -> p kt h", h=2)
            va3 = va.rearrange("p (kt x) -> p kt x", kt=KT)
            for h in range(2):
                nc.scalar.activation(va3[:, :, 32 + 33 * h:33 + 33 * h],
                                     lnv3[:, :, h:h + 1], AF.Exp, bias=0.0, scale=0.5)

            # ---------------- P4: k exp scales (column form) ----------------
            ksq = scr.tile([64, HW], BF16, tag="qksq", name="ksq")
            nc.gpsimd.tensor_mul(ksq, kT, kT)
            psk = scps.tile([128, 64], F32, tag="sc", name="psk")
            for kt in range(KT):
                for h in range(2):
                    nc.tensor.matmul(psk[:, kt * 2 + h:kt * 2 + h + 1],
                                     ksq[32 * h:32 * h + 32, kt * 128:kt * 128 + 128],
                                     ones_bf[32 * h:32 * h + 32, 0:1],
                                     start=True, stop=True)
            lnk = work.tile([128, 64], F32, tag="lnk")
            nc.scalar.activation(lnk, psk, AF.Ln, bias=eps_col[:, 0:1], scale=1.0 / HD)
            nc.scalar.activation(skcol, lnk, AF.Exp, bias=lniq_col[:, 0:1], scale=-0.5)
            # Schraudolph per-partition scale/bias for the DVE exp units
            nc.vector.tensor_scalar_mul(skS16, skcol, S16)
            nc.vector.tensor_scalar(vbB16, vbias, S16, B16, op0=ALU.mult, op1=ALU.add)

            # ---------------- P5: q pixel-norm multiply (row-form sums) -----
            # Row-form sums avoid the slow small-element transpose DMA bounce:
            # sum_d q^2 for 512 pixels lands as one [1,512] psum row at
            # partition 32r; ln/exp then run on [97,512] tiles, and the bch
            # broadcasts consume the resulting bf16 rows directly.
            qsq = scr.tile([64, HW], BF16, tag="qksq", name="qsq")
            nc.gpsimd.tensor_mul(qsq, qT, qT)
            sq4 = []
            for tb in range(4):
                p4 = scps.tile([128, 512], F32, tag="sc", name=f"psq4_{tb}")
                for r in range(4):
                    g = tb * 4 + r
                    h, ch = g // 8, g % 8
                    nc.tensor.matmul(p4[32 * r:32 * r + 1, :],
                                     ones_bf[32 * h:32 * h + 32, 0:1],
                                     qsq[32 * h:32 * h + 32, ch * 512:ch * 512 + 512],
                                     start=True, stop=True,
                                     tile_position=(32 * h, 32 * r))
                lnq4 = work.tile([97, 512], F32, tag="lnq4", name=f"lnq4_{tb}")
                nc.scalar.activation(lnq4, p4[0:97, :], AF.Ln,
                                     bias=eps_col[0:97, 0:1], scale=1.0 / HD)
                s4 = scr.tile([97, 512], BF16, tag="sq4", name=f"sq4_{tb}", bufs=4)
                nc.scalar.activation(s4, lnq4, AF.Exp, bias=0.0, scale=-0.5)
                sq4.append(s4)
            for ch in range(NCH):
                sl = slice(ch * 512, ch * 512 + 512)
                bch = scps.tile([128, 512], F32, tag="sc", name=f"bcq_{ch}")
                for h in range(2):
                    g = h * 8 + ch
                    tb, r = g // 4, 32 * (g % 4)
                    nc.tensor.matmul(bch[32 * h:32 * h + 32, :],
                                     ones_bf[r:r + 1, 0:32],
                                     sq4[tb][r:r + 1, :],
                                     start=True, stop=True,
                                     tile_position=(r, 32 * h))
                nc.vector.tensor_mul(qT[0:32, sl], qT[0:32, sl], bch[0:32, :])
                nc.vector.tensor_mul(qT[32:64, sl], qT[32:64, sl], bch[32:64, :])

            # ---------------- P6: attention ----------------
            uctr = [0]   # exp unit counter for the ACT/DVE split

            def emit_exp(sc_t, col, dst):
                u = uctr[0]
                uctr[0] += 1
                use_act = ((u + 1) * ACT_NUM) // ACT_DEN - (u * ACT_NUM) // ACT_DEN
                if use_act:
                    nc.scalar.activation(dst, sc_t, AF.Exp,
                                         bias=vbias[:, col:col + 1],
                                         scale=skcol[:, col:col + 1])
                else:
                    nc.vector.tensor_scalar(dst.bitcast(I16), sc_t,
                                            skS16[:, col:col + 1],
                                            vbB16[:, col:col + 1],
                                            op0=ALU.mult, op1=ALU.add)

            fin_steps = []   # deferred finalize closures from the previous pr

            def make_finalize(pr, acc, qcs):
                rden = {}
                yfin = {}

                def drain(j):
                    def go():
                        a = acc[j]
                        rd = work.tile([128, 512], BF16, tag="rden",
                                       name=f"rden{pr}_{j}")
                        with nc.allow_low_precision(reason="softmax denom recip"):
                            nc.vector.reciprocal(rd[32:33, :], a[32:33, :])
                            nc.vector.reciprocal(rd[96:97, :], a[96:97, :])
                        yf = op.tile([128, 512], F32, tag="yfin",
                                     name=f"yfin{pr}_{j}")
                        nc.vector.tensor_copy(yf[0:32, :], a[0:32, :])
                        nc.vector.tensor_copy(yf[64:96, :], a[64:96, :])
                        rden[j], yfin[j] = rd, yf
                    return go

                def bcnorm(j):
                    def go():
                        rd, yf = rden[j], yfin[j]
                        yfb = op.tile([128, 512], BF16, tag="yfb",
                                      name=f"yfb{pr}_{j}")
                        bc2a = scps.tile([128, 512], F32, tag="sc",
                                         name=f"bc2a_{pr}_{j}")
                        bc2b = scps.tile([128, 512], F32, tag="sc",
                                         name=f"bc2b_{pr}_{j}")
                        nc.tensor.matmul(bc2a[0:32, :], ones_bf[32:33, 0:32],
                                         rd[32:33, :], start=True, stop=True,
                                         tile_position=(32, 0))
                        nc.tensor.matmul(bc2b[64:96, :], ones_bf[96:97, 0:32],
                                         rd[96:97, :], start=True, stop=True,
                                         tile_position=(96, 64))
                        nc.vector.tensor_mul(yfb[0:32, :], yf[0:32, :], bc2a[0:32, :])
                        nc.vector.tensor_mul(yfb[64:96, :], yf[64:96, :], bc2b[64:96, :])
                        yfin[j] = yfb
                    return go

                def outconv(j, mt):
                    def go():
                        qc = qcs[j]
                        yfb = yfin[j]
                        opsa = scps.tile([128, 512], F32, tag="sc",
                                         name=f"opsa{pr}_{j}{mt}")
                        opsb = scps.tile([128, 512], F32, tag="sc",
                                         name=f"opsb{pr}_{j}{mt}")
                        nc.tensor.matmul(opsa, woT4_bf[0:32, mt * 128:mt * 128 + 128],
                                         yfb[0:32, :], start=True, stop=True)
                        nc.tensor.matmul(opsb, woT4_bf[64:96, mt * 128:mt * 128 + 128],
                                         yfb[64:96, :], start=True, stop=True)
                        osb = op.tile([128, 512], F32, tag="osb",
                                      name=f"osb{pr}_{j}{mt}")
                        nc.vector.scalar_tensor_tensor(
                            osb, opsa, swo[:, mt:mt + 1],
                            x_sb[:, mt, qc * 512:qc * 512 + 512], ALU.mult, ALU.add)
                        nc.vector.scalar_tensor_tensor(
                            osb, opsb, swo[:, mt:mt + 1], osb, ALU.mult, ALU.add)
                        nc.sync.dma_start(
                            out=y_d[:].rearrange("(t p) f -> p t f", p=128)[:, mt:mt + 1, qc * 512:qc * 512 + 512],
                            in_=osb)
                    return go

                return [drain(0), drain(1), bcnorm(0), outconv(0, 0), outconv(0, 1),
                        bcnorm(1), outconv(1, 0), outconv(1, 1)]

            for pr in range(NCH // 2):
                qcs = (2 * pr, 2 * pr + 1)
                acc = {}
                for j in range(2):
                    acc[j] = accps.tile([128, 512], F32, tag="acc",
                                        name=f"acc{pr}_{j}")
                sc = {}
                e = {}
                # software-pipelined kt loop: MM1(kt) || exp(kt) || MM2(kt-1)
                for kt in range(KT + 1):
                    if fin_steps:
                        fin_steps.pop(0)()
                    if kt < KT:
                        for h in range(2):
                            sc[(kt, h)] = scps.tile([128, 1024], F32, tag="sc",
                                                    name=f"sc{pr}_{kt}_{h}")
                        for j in range(2):
                            for h in range(2):   # h fastest: LDW row-groups alternate
                                nc.tensor.matmul(
                                    sc[(kt, h)][:, j * 512:j * 512 + 512],
                                    kT[32 * h:32 * h + 32, kt * 128:kt * 128 + 128],
                                    qT[32 * h:32 * h + 32,
                                       qcs[j] * 512:qcs[j] * 512 + 512],
                                    start=True, stop=True)
                        for h in range(2):
                            eh = ep.tile([128, 1024], BF16, tag="e",
                                         name=f"e{pr}_{kt}_{h}")
                            emit_exp(sc[(kt, h)], kt * 2 + h, eh)
                            e[(kt, h)] = eh
                    if kt >= 1:
                        kp = kt - 1
                        st, sp = kp == 0, kp == KT - 1
                        for j in range(2):
                            for h in range(2):   # h adjacent: disjoint col groups
                                lhs = va[:, kp * 66 + 33 * h:kp * 66 + 33 * h + 33]
                                ofs = 64 * h
                                nc.tensor.matmul(
                                    acc[j][ofs:ofs + 33, :], lhs,
                                    e[(kp, h)][:, j * 512:j * 512 + 512],
                                    start=st, stop=sp, tile_position=(0, ofs),
                                    skip_group_check=True)
                        e.pop((kp, 0)), e.pop((kp, 1))
                fin_steps = make_finalize(pr, acc, qcs)
            for s in fin_steps:
                s()

    if split:
        _split_waits(nc)
    return nc


_PROG = None
last_results = None


def kernel(x, w_qkv, w_out, num_heads):
    global _PROG
    x = np.asarray(x, dtype=np.float32)
    W = np.asarray(w_qkv, dtype=np.float32)[:, :, 0, 0]
    WO = np.asarray(w_out, dtype=np.float32)[:, :, 0, 0]
    b_, c_, hh, ww = x.shape
    assert (b_, c_, hh * ww) == (2, C, HW)

    if _PROG is None:
        _PROG = build_program()
    nc = _PROG

    in_maps = []
    for core in range(8):
        b = core // 4
        h0 = 2 * (core % 4)
        h1 = h0 + 1
        rq = np.concatenate([W[h0 * HD:(h0 + 1) * HD], W[h1 * HD:(h1 + 1) * HD]], 0)
        rk = np.concatenate([W[C + h0 * HD:C + (h0 + 1) * HD],
                             W[C + h1 * HD:C + (h1 + 1) * HD]], 0)
        rv = np.concatenate([W[2 * C + h0 * HD:2 * C + (h0 + 1) * HD],
                             W[2 * C + h1 * HD:2 * C + (h1 + 1) * HD]], 0)
        woT4 = np.zeros((128, C), np.float32)
        woT4[0:32] = WO[:, h0 * HD:(h0 + 1) * HD].T
        woT4[64:96] = WO[:, h1 * HD:(h1 + 1) * HD].T
        in_maps.append({
            "x": np.ascontiguousarray(x[b].reshape(C, HW)),
            "wqn": np.ascontiguousarray(rq),
            "wkn": np.ascontiguousarray(rk),
            "wqT": np.ascontiguousarray(rq.T),
            "wkT": np.ascontiguousarray(rk.T),
            "wvT": np.ascontiguousarray(rv.T),
            "won": np.ascontiguousarray(WO),
            "woT4": woT4,
        })

    res = run_bass_kernel_spmd(nc, in_maps, list(range(8)))
    global last_results
    last_results = res
    outs = [r["y"] for r in res.results]
    full = np.zeros((2, C, HW), np.float32)
    for core in range(8):
        full[core // 4] += outs[core]
    return full.reshape(b_, c_, hh, ww)


# revision 9
# speedup vs baseline: 3.9456x; 1.1511x over previous
"""CosineAttention Trainium2 kernel (8 NeuronCores, SPMD).

Sharding: 16 (batch, head) pairs -> 8 cores, 2 heads (one batch) per core.
Per core, attention runs in transposed-score layout (scoresT[kpos, qpos]) so
both attention matmuls contract over the partition dim with no transposes:
  MM1: scoresT = kT.T-slice.T @ qT-slice, issued as 8 concurrent 32x32 PE
       tiles per (kt, q-chunk) so the K=32 contraction doesn't idle the array
  exp: e = sv[k]*exp(true scoreT), per-partition scale=(k pixel-norm /
       sqrt(hd)) and bias=ln(sv) folded in. Split ~8/13 ACT (exact Exp) and
       ~5/13 DVE (Schraudolph int16 bit-trick producing bf16).
  MM2: lhsT = [v_hat | 1/sv] (M=33): accumulates y^T rows and the softmax
       denominator row in one PSUM accumulation group; h0/h1 share one bank
       per q-chunk (rows 0:33 / 64:97) and run as concurrent col-tiles.
Attention matmul operands are bf16 (1 col/cycle on the PE vs 4 for fp32).
The kt-loop is software-pipelined (MM1(kt) | exp(kt) | MM2(kt-1)) with a
3-deep score ring, and each pr's finalize is spread into the next pr's kt
slots so the PE queue never blocks on the DVE finalize chain. The softmax
denominator reciprocal uses a bf16 magic-constant bit trick (one int16 DVE
op instead of a 3.3us iterative reciprocal).
The out-conv is computed per-core on the core's 64 attention channels; the
8 partial results (each including 1/4 of the x residual term, via the
host-prescaled xs input) are summed on host per batch.
"""
import numpy as np
import concourse.bass as bass
import concourse.tile as tile
from concourse import mybir
from concourse.bass_utils import run_bass_kernel_spmd

F32 = mybir.dt.float32
BF16 = mybir.dt.bfloat16
I16 = mybir.dt.int16
AF = mybir.ActivationFunctionType
ALU = mybir.AluOpType

EPS = 1e-4
MP_T = 0.3
INV_SCALE = 1.0 / np.sqrt(MP_T ** 2 + (1.0 - MP_T) ** 2)
C = 256          # channels
HW = 4096        # pixels
HD = 32          # head dim
NCH = 8          # 512-wide pixel chunks
KT = 32          # 128-wide kpos tiles
LOG_ISQ_HD = float(np.log(1.0 / np.sqrt(HD)))
C_X = 0.25 * (1.0 - MP_T) * INV_SCALE     # per-core share of residual
C_Y = MP_T * INV_SCALE                    # folded into w_out scale
W_EPS = 16.0 * EPS                        # sqrt(fan_in)*EPS with fan_in=256
S16 = float(2 ** 7 / np.log(2.0))         # Schraudolph bf16 scale
B16 = float(127 * 2 ** 7 - 7.42)          # Schraudolph bf16 bias (min max-err)
RCP_MAGIC = 0x7EF2                        # bf16 reciprocal magic constant
ACT_NUM, ACT_DEN = 8, 13                  # ~61.5% of exp units on ACT


def _split_waits(nc):
    """This walrus accepts 1 sync wait per engine instruction: hoist extras
    into preceding NoOps on the same engine (engines are in-order)."""
    for f in nc.m.functions:
        for bb in f.blocks:
            newlist = []
            for inst in bb.instructions:
                si = inst.sync_info
                if si is not None and si.on_wait is not None and len(si.on_wait) > 1:
                    waits = list(si.on_wait)
                    if "DMA" in type(inst).__name__:
                        # keep the compute-engine sem on the DMA descriptor;
                        # hoist DMA-queue sems (monotonic, engine-stall safe)
                        hw = [w for w in waits if str(w.ant_name).startswith("DMA")]
                        eng = [w for w in waits if not str(w.ant_name).startswith("DMA")]
                        if eng:
                            keep, extra = eng[-1:], hw + eng[:-1]
                        else:
                            keep, extra = hw[-1:], hw[:-1]
                    else:
                        extra, keep = waits[:-1], waits[-1:]
                    for idx, w in enumerate(extra):
                        nop = mybir.InstNoOp(
                            name=f"{inst.name}_ws{idx}", ins=[], outs=[],
                            sync_info=mybir.SyncInfo(on_wait=[w], on_update=[]))
                        nop.engine = inst.engine
                        newlist.append(nop)
                    inst.sync_info = mybir.SyncInfo(
                        on_wait=keep, on_update=list(si.on_update or []))
                newlist.append(inst)
            bb.instructions = newlist


def _weight_scale_rows(nc, work, nat_ap, p):
    """Per-row weight-norm scale s = 1/(||w_row|| + 16*eps) for natural-layout
    [p, 256] weight rows. Returns a [p, 1] sbuf AP."""
    sq = work.tile([p, 256], F32, tag="wsq", name=f"wsq_{nc.next_id()}")
    nc.vector.tensor_mul(sq, nat_ap, nat_ap)
    ssq = work.tile([p, 1], F32, tag="wssq", name=f"wssq_{nc.next_id()}")
    nc.vector.tensor_reduce(ssq, sq, axis=mybir.AxisListType.X, op=ALU.add)
    ln = work.tile([p, 1], F32, tag="wln", name=f"wln_{nc.next_id()}")
    nc.scalar.activation(ln, ssq, AF.Ln, bias=0.0, scale=1.0)
    n = work.tile([p, 1], F32, tag="wn", name=f"wn_{nc.next_id()}")
    nc.scalar.activation(n, ln, AF.Exp, bias=0.0, scale=0.5)
    ne = work.tile([p, 1], F32, tag="wne", name=f"wne_{nc.next_id()}")
    nc.vector.tensor_scalar_add(ne, n, W_EPS)
    s = work.tile([p, 1], F32, tag="ws", name=f"ws_{nc.next_id()}")
    nc.vector.reciprocal(s, ne)
    return s


def build_program(split=True):
    nc = bass.Bass()
    x_d = nc.declare_dram_parameter("x", [C, HW], F32, isOutput=False)
    xs_d = nc.declare_dram_parameter("xs", [C, HW], F32, isOutput=False)
    wqn_d = nc.declare_dram_parameter("wqn", [64, C], F32, isOutput=False)
    wkn_d = nc.declare_dram_parameter("wkn", [64, C], F32, isOutput=False)
    wqT_d = nc.declare_dram_parameter("wqT", [C, 64], F32, isOutput=False)
    wkT_d = nc.declare_dram_parameter("wkT", [C, 64], F32, isOutput=False)
    wvT_d = nc.declare_dram_parameter("wvT", [C, 64], F32, isOutput=False)
    won_d = nc.declare_dram_parameter("won", [C, C], F32, isOutput=False)
    woT4_d = nc.declare_dram_parameter("woT4", [128, C], F32, isOutput=False)
    y_d = nc.declare_dram_parameter("y", [C, HW], F32, isOutput=True)

    with tile.TileContext(nc) as tc:
        with tc.tile_pool(name="singles", bufs=1) as sg, \
             tc.tile_pool(name="work", bufs=2) as work, \
             tc.tile_pool(name="scratch", bufs=2) as scr, \
             tc.tile_pool(name="epool", bufs=4) as ep, \
             tc.tile_pool(name="opool", bufs=4) as op, \
             tc.tile_pool(name="scps", bufs=3, space="PSUM") as scps, \
             tc.tile_pool(name="accps", bufs=2, space="PSUM") as accps:

            # ---------------- P0: loads ----------------
            xr = x_d[:].rearrange("(t p) f -> p t f", p=128)
            x_t = []
            for ch in range(NCH):
                xt = sg.tile([128, 2, 512], F32, name=f"x_t{ch}")
                nc.sync.dma_start(out=xt, in_=xr[:, :, ch * 512:ch * 512 + 512])
                x_t.append(xt)
            xs_sb = sg.tile([128, 2, HW], F32)
            nc.sync.dma_start(out=xs_sb, in_=xs_d[:].rearrange("(t p) f -> p t f", p=128))
            wqT_sb = sg.tile([128, 2, 64], F32)
            nc.sync.dma_start(out=wqT_sb, in_=wqT_d[:].rearrange("(t p) m -> p t m", p=128))
            wkT_sb = sg.tile([128, 2, 64], F32)
            nc.sync.dma_start(out=wkT_sb, in_=wkT_d[:].rearrange("(t p) m -> p t m", p=128))
            wvT_sb = sg.tile([128, 2, 64], F32)
            nc.sync.dma_start(out=wvT_sb, in_=wvT_d[:].rearrange("(t p) m -> p t m", p=128))
            wqn_sb = sg.tile([64, C], F32)
            nc.sync.dma_start(out=wqn_sb, in_=wqn_d[:])
            wkn_sb = sg.tile([64, C], F32)
            nc.sync.dma_start(out=wkn_sb, in_=wkn_d[:])
            won_sb = sg.tile([128, 2, C], F32)
            nc.sync.dma_start(out=won_sb, in_=won_d[:].rearrange("(t p) m -> p t m", p=128))
            woT4_sb = sg.tile([128, C], F32)
            nc.sync.dma_start(out=woT4_sb, in_=woT4_d[:])
            ones = sg.tile([128, 128], F32)
            nc.vector.memset(ones, 1.0)
            ones_bf = sg.tile([128, 64], BF16)
            nc.vector.memset(ones_bf, 1.0)
            eps_col = sg.tile([128, 1], F32)
            nc.vector.memset(eps_col, EPS)
            lniq_col = sg.tile([128, 1], F32)
            nc.vector.memset(lniq_col, LOG_ISQ_HD)

            woT4_bf = sg.tile([128, C], BF16)
            nc.vector.tensor_copy(woT4_bf, woT4_sb)

            qT = sg.tile([64, HW], BF16)
            kT = sg.tile([64, HW], BF16)
            va = sg.tile([128, KT * 66], BF16)  # per kt: v_h0(32)|sinv_h0|v_h1(32)|sinv_h1
            vbias = sg.tile([128, 64], F32)     # ln(sv), col = kt*2 + h
            skcol = sg.tile([128, 64], F32)     # exp scale, col = kt*2 + h
            skS16 = sg.tile([128, 64], F32)     # Schraudolph scale = skcol * S16
            vbB16 = sg.tile([128, 64], F32)     # Schraudolph bias = vbias*S16 + B16

            # ---------------- P1: weight-norm scales ----------------
            sqq = _weight_scale_rows(nc, work, wqn_sb[:, :], 64)
            sqk = _weight_scale_rows(nc, work, wkn_sb[:, :], 64)
            swo = sg.tile([128, 2], F32)
            wosq = work.tile([128, 2, C], F32, tag="wosq")
            nc.vector.tensor_mul(wosq, won_sb, won_sb)
            wossq = work.tile([128, 2], F32, tag="wossq")
            nc.vector.tensor_reduce(wossq, wosq, axis=mybir.AxisListType.X, op=ALU.add)
            woln = work.tile([128, 2], F32, tag="woln")
            nc.scalar.activation(woln, wossq, AF.Ln, bias=0.0, scale=1.0)
            won_n = work.tile([128, 2], F32, tag="won_n")
            nc.scalar.activation(won_n, woln, AF.Exp, bias=0.0, scale=0.5)
            won_ne = work.tile([128, 2], F32, tag="won_ne")
            nc.vector.tensor_scalar_add(won_ne, won_n, W_EPS)
            swo_inv = work.tile([128, 2], F32, tag="swo_inv")
            nc.vector.reciprocal(swo_inv, won_ne)
            nc.vector.tensor_scalar_mul(swo, swo_inv, float(C_Y))

            # wv column scales: s_v[col] = 1/(||w_v[col]|| + 16 eps), fold into wvT
            wvsq = work.tile([128, 2, 64], F32, tag="wvsq")
            nc.vector.tensor_mul(wvsq, wvT_sb, wvT_sb)
            ssqv_ps = scps.tile([1, 64], F32, tag="sc", name="ssqv_ps")
            for t in range(2):
                nc.tensor.matmul(ssqv_ps, ones[:, 0:1], wvsq[:, t, :],
                                 start=(t == 0), stop=(t == 1))
            vln = work.tile([1, 64], F32, tag="vln")
            nc.scalar.activation(vln, ssqv_ps, AF.Ln, bias=0.0, scale=1.0)
            vn = work.tile([1, 64], F32, tag="vn")
            nc.scalar.activation(vn, vln, AF.Exp, bias=0.0, scale=0.5)
            vne = work.tile([1, 64], F32, tag="vne")
            nc.vector.tensor_scalar_add(vne, vn, W_EPS)
            svrow = work.tile([1, 64], F32, tag="svrow")
            nc.vector.reciprocal(svrow, vne)
            svbc_ps = scps.tile([128, 64], F32, tag="sc", name="svbc_ps")
            nc.tensor.matmul(svbc_ps, ones[0:1, 0:128], svrow[0:1, :],
                             start=True, stop=True)
            for t in range(2):
                nc.vector.tensor_mul(wvT_sb[:, t, :], wvT_sb[:, t, :], svbc_ps)

            # ---------------- P2: convs (fp32, exact pre-norm stats) --------
            for ch in range(NCH):
                sl = slice(ch * 512, ch * 512 + 512)
                pq = scps.tile([128, 512], F32, tag="sc", name=f"pq{ch}")
                pk = scps.tile([128, 512], F32, tag="sc", name=f"pk{ch}")
                for t in range(2):
                    nc.tensor.matmul(pq[0:64, :], wqT_sb[:, t, :],
                                     x_t[ch][:, t, :], start=(t == 0), stop=(t == 1))
                for t in range(2):
                    nc.tensor.matmul(pk[0:64, :], wkT_sb[:, t, :],
                                     x_t[ch][:, t, :], start=(t == 0), stop=(t == 1))
                nc.vector.tensor_scalar(qT[:, sl], pq[0:64, :], sqq[:, 0:1],
                                        None, op0=ALU.mult)
                nc.vector.tensor_scalar(kT[:, sl], pk[0:64, :], sqk[:, 0:1],
                                        None, op0=ALU.mult)
            va_v = va.rearrange("p (kt x) -> p kt x", kt=KT)
            for g in range(NCH):
                pvg = scps.tile([128, 256], F32, tag="sc", name=f"pvg{g}")
                for q in range(4):
                    pt = g * 4 + q
                    for t in range(2):
                        nc.tensor.matmul(pvg[:, q * 64:q * 64 + 64],
                                         x_t[g][:, t, q * 128:q * 128 + 128],
                                         wvT_sb[:, t, :], start=(t == 0), stop=(t == 1))
                pvg4 = pvg.rearrange("p (q e) -> p q e", q=4)
                for h in range(2):
                    nc.vector.tensor_copy(
                        va_v[:, g * 4:g * 4 + 4, 33 * h:33 * h + 32],
                        pvg4[:, :, 32 * h:32 * h + 32])

            # ---------------- P3: v pixel-norm stats ----------------
            va4 = va.rearrange("p (kt h e) -> p kt h e", kt=KT, h=2)
            vsq = scr.tile([128, KT, 2, HD], F32, tag="vsq")
            nc.vector.tensor_mul(vsq, va4[:, :, :, 0:HD], va4[:, :, :, 0:HD])
            msum = sg.tile([128, 64], F32)
            nc.vector.tensor_reduce(msum, vsq, axis=mybir.AxisListType.X, op=ALU.add)
            lnv = sg.tile([128, 64], F32)
            nc.scalar.activation(lnv, msum, AF.Ln, bias=eps_col[:, 0:1], scale=1.0 / HD)
            nc.vector.tensor_scalar_mul(vbias, lnv, -0.5)
            lnv3 = lnv.rearrange("p (kt h) -> p kt h", h=2)
            va3 = va.rearrange("p (kt x) -> p kt x", kt=KT)
            for h in range(2):
                nc.scalar.activation(va3[:, :, 32 + 33 * h:33 + 33 * h],
                                     lnv3[:, :, h:h + 1], AF.Exp, bias=0.0, scale=0.5)

            # ---------------- P4: k exp scales (column form) ----------------
            ksq = scr.tile([64, HW], BF16, tag="qksq", name="ksq")
            nc.vector.tensor_mul(ksq, kT, kT)
            psk = scps.tile([128, 64], F32, tag="sc", name="psk")
            for kt in range(KT):
                for h in range(2):
                    nc.tensor.matmul(psk[:, kt * 2 + h:kt * 2 + h + 1],
                                     ksq[32 * h:32 * h + 32, kt * 128:kt * 128 + 128],
                                     ones_bf[32 * h:32 * h + 32, 0:1],
                                     start=True, stop=True)
            lnk = work.tile([128, 64], F32, tag="lnk")
            nc.scalar.activation(lnk, psk, AF.Ln, bias=eps_col[:, 0:1], scale=1.0 / HD)
            nc.scalar.activation(skcol, lnk, AF.Exp, bias=lniq_col[:, 0:1], scale=-0.5)
            # Schraudolph per-partition scale/bias for the DVE exp units
            nc.vector.tensor_scalar_mul(skS16, skcol, S16)
            nc.vector.tensor_scalar(vbB16, vbias, S16, B16, op0=ALU.mult, op1=ALU.add)

            # ---------------- P5: q pixel-norm multiply (row-form sums) -----
            qsq = scr.tile([64, HW], BF16, tag="qksq", name="qsq")
            nc.vector.tensor_mul(qsq, qT, qT)
            sq4 = []
            for tb in range(4):
                p4 = scps.tile([128, 512], F32, tag="sc", name=f"psq4_{tb}")
                for r in range(4):
                    g = tb * 4 + r
                    h, ch = g // 8, g % 8
                    nc.tensor.matmul(p4[32 * r:32 * r + 1, :],
                                     ones_bf[32 * h:32 * h + 32, 0:1],
                                     qsq[32 * h:32 * h + 32, ch * 512:ch * 512 + 512],
                                     start=True, stop=True,
                                     tile_position=(32 * h, 32 * r))
                lnq4 = work.tile([97, 512], F32, tag="lnq4", name=f"lnq4_{tb}")
                nc.scalar.activation(lnq4, p4[0:97, :], AF.Ln,
                                     bias=eps_col[0:97, 0:1], scale=1.0 / HD)
                s4 = scr.tile([97, 512], BF16, tag="sq4", name=f"sq4_{tb}", bufs=4)
                nc.scalar.activation(s4, lnq4, AF.Exp, bias=0.0, scale=-0.5)
                sq4.append(s4)
            for ch in range(NCH):
                sl = slice(ch * 512, ch * 512 + 512)
                bch = scps.tile([128, 512], F32, tag="sc", name=f"bcq_{ch}")
                for h in range(2):
                    g = h * 8 + ch
                    tb, r = g // 4, 32 * (g % 4)
                    nc.tensor.matmul(bch[32 * h:32 * h + 32, :],
                                     ones_bf[r:r + 1, 0:32],
                                     sq4[tb][r:r + 1, :],
                                     start=True, stop=True,
                                     tile_position=(r, 32 * h))
                nc.vector.tensor_mul(qT[0:32, sl], qT[0:32, sl], bch[0:32, :])
                nc.vector.tensor_mul(qT[32:64, sl], qT[32:64, sl], bch[32:64, :])

            # ---------------- P6: attention ----------------
            uctr = [0]   # exp unit counter for the ACT/DVE split

            def emit_exp(sc_t, col, dst):
                u = uctr[0]
                uctr[0] += 1
                use_act = ((u + 1) * ACT_NUM) // ACT_DEN - (u * ACT_NUM) // ACT_DEN
                if use_act:
                    nc.scalar.activation(dst, sc_t, AF.Exp,
                                         bias=vbias[:, col:col + 1],
                                         scale=skcol[:, col:col + 1])
                else:
                    nc.vector.tensor_scalar(dst.bitcast(I16), sc_t,
                                            skS16[:, col:col + 1],
                                            vbB16[:, col:col + 1],
                                            op0=ALU.mult, op1=ALU.add)

            fin_steps = []   # deferred finalize closures from the previous pr

            def make_finalize(pr, acc, qcs):
                rden = {}
                yfin = {}

                def drain(j):
                    def go():
                        a = acc[j]
                        rd = work.tile([128, 512], BF16, tag="rden",
                                       name=f"rden{pr}_{j}")
                        with nc.allow_low_precision(reason="softmax denom recip"):
                            nc.vector.reciprocal(rd[32:33, :], a[32:33, :])
                            nc.vector.reciprocal(rd[96:97, :], a[96:97, :])
                        rden[j] = rd
                        # one free-size-bound copy drains rows 0..96 (frees the
                        # acc bank before the next pr's first MM2 in PE order)
                        yf = op.tile([128, 512], F32, tag="yfin",
                                     name=f"yfin{pr}_{j}")
                        nc.vector.tensor_copy(yf[0:97, :], a[0:97, :])
                        yfin[j] = yf
                    return go

                def bcnorm(j):
                    def go():
                        rd = rden[j]
                        yf = yfin[j]
                        yfb = op.tile([128, 512], BF16, tag="yfb",
                                      name=f"yfb{pr}_{j}")
                        bc2 = scps.tile([128, 512], F32, tag="sc",
                                        name=f"bc2_{pr}_{j}")
                        nc.tensor.matmul(bc2[0:32, :], ones_bf[32:33, 0:32],
                                         rd[32:33, :], start=True, stop=True,
                                         tile_position=(32, 0))
                        nc.tensor.matmul(bc2[64:96, :], ones_bf[96:97, 0:32],
                                         rd[96:97, :], start=True, stop=True,
                                         tile_position=(96, 64))
                        nc.vector.tensor_mul(yfb[0:97, :], yf[0:97, :], bc2[0:97, :])
                        yfin[j] = yfb
                    return go

                def outconv(j, mt):
                    def go():
                        qc = qcs[j]
                        yfb = yfin[j]
                        opsa = scps.tile([128, 512], F32, tag="sc",
                                         name=f"opsa{pr}_{j}{mt}")
                        opsb = scps.tile([128, 512], F32, tag="sc",
                                         name=f"opsb{pr}_{j}{mt}")
                        nc.tensor.matmul(opsa, woT4_bf[0:32, mt * 128:mt * 128 + 128],
                                         yfb[0:32, :], start=True, stop=True)
                        nc.tensor.matmul(opsb, woT4_bf[64:96, mt * 128:mt * 128 + 128],
                                         yfb[64:96, :], start=True, stop=True)
                        osb = op.tile([128, 512], F32, tag="osb",
                                      name=f"osb{pr}_{j}{mt}")
                        nc.vector.scalar_tensor_tensor(
                            osb, opsa, swo[:, mt:mt + 1],
                            xs_sb[:, mt, qc * 512:qc * 512 + 512], ALU.mult, ALU.add)
                        nc.vector.scalar_tensor_tensor(
                            osb, opsb, swo[:, mt:mt + 1], osb, ALU.mult, ALU.add)
                        nc.sync.dma_start(
                            out=y_d[:].rearrange("(t p) f -> p t f", p=128)[:, mt:mt + 1, qc * 512:qc * 512 + 512],
                            in_=osb)
                    return go

                return [drain(0), drain(1), bcnorm(0), outconv(0, 0), bcnorm(1),
                        outconv(0, 1), outconv(1, 0), outconv(1, 1)]

            for pr in range(NCH // 2):
                qcs = (2 * pr, 2 * pr + 1)
                acc = {}
                for j in range(2):
                    acc[j] = accps.tile([128, 512], F32, tag="acc",
                                        name=f"acc{pr}_{j}")
                sc = {}
                e = {}
                # software-pipelined kt loop: MM1(kt) || exp(kt) || MM2(kt-1)
                for kt in range(KT + 1):
                    if fin_steps:
                        fin_steps.pop(0)()
                    if kt < KT:
                        for h in range(2):
                            sc[(kt, h)] = scps.tile([128, 1024], F32, tag="sc",
                                                    name=f"sc{pr}_{kt}_{h}")
                        for j in range(2):
                            for cg in range(4):
                                for h in range(2):   # adjacent instrs: disjoint tiles
                                    nc.tensor.matmul(
                                        sc[(kt, h)][32 * cg:32 * cg + 32,
                                                    j * 512:j * 512 + 512],
                                        kT[32 * h:32 * h + 32,
                                           kt * 128 + 32 * cg:kt * 128 + 32 * cg + 32],
                                        qT[32 * h:32 * h + 32,
                                           qcs[j] * 512:qcs[j] * 512 + 512],
                                        start=True, stop=True,
                                        tile_position=(32 * h, 32 * cg))
                        for h in range(2):
                            eh = ep.tile([128, 1024], BF16, tag="e",
                                         name=f"e{pr}_{kt}_{h}")
                            emit_exp(sc[(kt, h)], kt * 2 + h, eh)
                            e[(kt, h)] = eh
                    if kt >= 1:
                        kp = kt - 1
                        st, sp = kp == 0, kp == KT - 1
                        for j in range(2):
                            for h in range(2):   # h adjacent: disjoint col groups
                                lhs = va[:, kp * 66 + 33 * h:kp * 66 + 33 * h + 33]
                                ofs = 64 * h
                                nc.tensor.matmul(
                                    acc[j][ofs:ofs + 33, :], lhs,
                                    e[(kp, h)][:, j * 512:j * 512 + 512],
                                    start=st, stop=sp, tile_position=(0, ofs),
                                    skip_group_check=True)
                        e.pop((kp, 0)), e.pop((kp, 1))
                fin_steps = make_finalize(pr, acc, qcs)
            for s in fin_steps:
                s()

    if split:
        _split_waits(nc)
    return nc


_PROG = None
last_results = None


def kernel(x, w_qkv, w_out, num_heads):
    global _PROG
    x = np.asarray(x, dtype=np.float32)
    W = np.asarray(w_qkv, dtype=np.float32)[:, :, 0, 0]
    WO = np.asarray(w_out, dtype=np.float32)[:, :, 0, 0]
    b_, c_, hh, ww = x.shape
    assert (b_, c_, hh * ww) == (2, C, HW)

    if _PROG is None:
        _PROG = build_program()
    nc = _PROG

    in_maps = []
    for core in range(8):
        b = core // 4
        h0 = 2 * (core % 4)
        h1 = h0 + 1
        rq = np.concatenate([W[h0 * HD:(h0 + 1) * HD], W[h1 * HD:(h1 + 1) * HD]], 0)
        rk = np.concatenate([W[C + h0 * HD:C + (h0 + 1) * HD],
                             W[C + h1 * HD:C + (h1 + 1) * HD]], 0)
        rv = np.concatenate([W[2 * C + h0 * HD:2 * C + (h0 + 1) * HD],
                             W[2 * C + h1 * HD:2 * C + (h1 + 1) * HD]], 0)
        woT4 = np.zeros((128, C), np.float32)
        woT4[0:32] = WO[:, h0 * HD:(h0 + 1) * HD].T
        woT4[64:96] = WO[:, h1 * HD:(h1 + 1) * HD].T
        xb = np.ascontiguousarray(x[b].reshape(C, HW))
        in_maps.append({
            "x": xb,
            "xs": np.ascontiguousarray(xb * np.float32(C_X)),
            "wqn": np.ascontiguousarray(rq),
            "wkn": np.ascontiguousarray(rk),
            "wqT": np.ascontiguousarray(rq.T),
            "wkT": np.ascontiguousarray(rk.T),
            "wvT": np.ascontiguousarray(rv.T),
            "won": np.ascontiguousarray(WO),
            "woT4": woT4,
        })

    res = run_bass_kernel_spmd(nc, in_maps, list(range(8)))
    global last_results
    last_results = res
    outs = [r["y"] for r in res.results]
    full = np.zeros((2, C, HW), np.float32)
    for core in range(8):
        full[core // 4] += outs[core]
    return full.reshape(b_, c_, hh, ww)
